# revision 23
# baseline (speedup 1.0000x reference)
"""Trainium2 8-core kernel for RMSNorm -> QKV -> RoPE -> causal SDPA -> out-proj.

Sharding: core c = b*4 + g handles batch b (of 2) and heads 4g..4g+3 (of 16).
Each core computes a partial out-projection [dim, tokens]; the host sums the
4 head-group partials per batch (the tensor-parallel "unshard") and adds b_o.

Cost-model-driven layout (TimelineSim charges matmuls by OUTPUT FREE SIZE
only — contraction depth and output partitions are free):
  - scores per (head, kb): [key 128, q free] trimmed to the causal triangle.
  - AV runs TRANSPOSED: out [q 128, d 65] so each accumulation step costs 65
    rows instead of ~512; the ones column gives the softmax denominator.
  - The normalized token-major AV result is returned to feature-major layout
    with DMA-engine transposes (14ns/32x32 tile, off the compute engines).
  - exp for a head PAIR is fused into one Activation instruction (the two
    heads' score tiles sit in adjacent PSUM banks).
  - r = rsqrt(mean x^2) rides into Q via r-scaled RoPE tables, into scores
    via the per-key `scale` operand of exp, and into V via a per-partition
    tensor_scalar during the PSUM->SBUF copy. r_tok (token-major r) comes
    from 16 free PE transposes of the r row.
  - PE is kept continuously busy (the cost model halves PE speed after any
    idle gap until 3us of continuous execution): the K projection starts at
    xT chunk 2 so the DMA stream stays ahead of the PE stream.
"""

import os

import numpy as np
import ml_dtypes

BF16 = ml_dtypes.bfloat16

DIM = 1024
HEADS = 16
DIM_HEAD = 64
T = 2048  # tokens per batch
B = 2
HPC = 4  # heads per core
F = HPC * DIM_HEAD  # 256 per-core head width
KC = DIM // 128  # 8 contraction chunks
KORD = [2, 3, 4, 5, 6, 7, 0, 1]  # kc order: first matmul waits for chunk 2

_NC_CACHE = {}


def _build_nc():
    import concourse.bacc as bacc
    import concourse.mybir as mybir
    import concourse.tile as tile
    from contextlib import ExitStack

    f32 = mybir.dt.float32
    bf16 = mybir.dt.bfloat16
    nc = bacc.Bacc()

    xT = nc.declare_dram_parameter("xT", [DIM, T], bf16, isOutput=False)
    wq = nc.declare_dram_parameter("wq", [DIM, F], bf16, isOutput=False)
    wk = nc.declare_dram_parameter("wk", [DIM, F], bf16, isOutput=False)
    wv = nc.declare_dram_parameter("wv", [DIM, F], bf16, isOutput=False)
    wo = nc.declare_dram_parameter("wo", [F, DIM], bf16, isOutput=False)
    cosT = nc.declare_dram_parameter("cosT", [128, T], bf16, isOutput=False)
    sinT = nc.declare_dram_parameter("sinT", [128, T], bf16, isOutput=False)
    perm = nc.declare_dram_parameter("perm", [128, 128], bf16, isOutput=False)
    masks = nc.declare_dram_parameter("masks", [128, 128], bf16, isOutput=False)
    ident = nc.declare_dram_parameter("ident", [128, 128], bf16, isOutput=False)
    out = nc.declare_dram_parameter("out", [DIM, T], bf16, isOutput=True)
    tap = os.environ.get("KTAP", "")
    dbg = None
    if tap:
        _tap_shapes = {
            "rtok": ([128, 16], f32),
            "qk": ([128, 4, T], bf16),
            "v": ([128, 16, HPC, 65], bf16),
            "avtok": ([128, 16, F], bf16),
            "avall": ([128, 2, T], bf16),
        }
        shp, dt = _tap_shapes[tap]
        dbg = nc.declare_dram_parameter("dbg", shp, dt, isOutput=True)

    Exp = mybir.ActivationFunctionType.Exp
    Sqrt = mybir.ActivationFunctionType.Sqrt
    mult = mybir.AluOpType.mult
    add = mybir.AluOpType.add

    with ExitStack() as ctx:
        tc = ctx.enter_context(tile.TileContext(nc))
        consts = ctx.enter_context(tc.tile_pool(name="consts", bufs=1))
        persist = ctx.enter_context(tc.tile_pool(name="persist", bufs=1))
        work = ctx.enter_context(tc.tile_pool(name="work", bufs=4))
        vecs = ctx.enter_context(tc.tile_pool(name="vecs", bufs=1))

        # ---- constants / inputs ----
        wk_sb = consts.tile([128, KC, F], bf16, tag="wk")
        wq_sb = consts.tile([128, KC, F], bf16, tag="wq")
        wv_sb = consts.tile([128, KC, F], bf16, tag="wv")
        wo_sb = consts.tile([128, 2, DIM], bf16, tag="wo")
        cos_sb = consts.tile([128, T], bf16, tag="cos")
        sin_sb = consts.tile([128, T], bf16, tag="sin")
        perm_sb = consts.tile([128, 128], bf16, tag="perm")
        mask_sb = consts.tile([128, 128], bf16, tag="mask")
        id_sb = consts.tile([128, 128], bf16, tag="ident")
        ones_col = consts.tile([128, 1], bf16, tag="onesc")
        one_f32 = consts.tile([1, 1], f32, tag="onef")
        xT_sb = persist.tile([128, KC, T], bf16, tag="xT")
        xT_r = xT.rearrange("(kc p) t -> p kc t", p=128)
        for kc in KORD:
            nc.sync.dma_start(xT_sb[:, kc], xT_r[:, kc])
        nc.sync.dma_start(wk_sb, wk.rearrange("(kc p) f -> p kc f", p=128))
        nc.sync.dma_start(cos_sb, cosT[:, :])
        nc.sync.dma_start(sin_sb, sinT[:, :])
        nc.sync.dma_start(perm_sb, perm[:, :])
        nc.sync.dma_start(wq_sb, wq.rearrange("(kc p) f -> p kc f", p=128))
        nc.sync.dma_start(wv_sb, wv.rearrange("(kc p) f -> p kc f", p=128))
        nc.sync.dma_start(mask_sb, masks[:, :])
        nc.sync.dma_start(id_sb, ident[:, :])
        nc.sync.dma_start(wo_sb, wo.rearrange("(fc p) d -> p fc d", p=128))
        nc.vector.memset(ones_col, 1.0)
        nc.vector.memset(one_f32, 1.0)

        # persistent activations
        qk_sb = persist.tile([128, 4, T], bf16, tag="qk")  # 0,1=q fc0/1; 2,3=k
        v_sb = persist.tile([128, 16, HPC, 65], bf16, tag="v")
        av_tok = persist.tile([128, 16, F], bf16, tag="avtok")
        av_all = persist.tile([128, 2, T], bf16, tag="av")
        r_sb = vecs.tile([1, T], f32, tag="r")
        r_tok = vecs.tile([128, 16], f32, tag="rtok")
        cosr_sb = persist.tile([128, T], bf16, tag="cosr")
        sinr_sb = persist.tile([128, T], bf16, tag="sinr")
        nc.vector.memset(v_sb[:, :, :, 64:65], 1.0)

        ctxA = ExitStack()
        psKQ = ctxA.enter_context(tc.tile_pool(name="psKQ", bufs=8, space="PSUM"))
        sbA = ctxA.enter_context(tc.tile_pool(name="sbA", bufs=1))
        xsq_sb = sbA.tile([128, KC, T], bf16, tag="xsq")

        # x^2 per chunk (DVE, chases the xT DMAs)
        for kc in KORD:
            nc.vector.tensor_mul(xsq_sb[:, kc], xT_sb[:, kc], xT_sb[:, kc])

        def proj_rope(fidx, psum_tiles, copy_scaled):
            """Finish a Q/K projection: PSUM->SBUF copy, rotate-half perm
            matmul, rope multiply-adds into qk_sb[fidx]."""
            cc = cosr_sb if copy_scaled else cos_sb
            ssb = sinr_sb if copy_scaled else sin_sb
            for tt in range(4):
                ts = slice(tt * 512, (tt + 1) * 512)
                raw = work.tile([128, 512], bf16, tag="raw")
                nc.vector.tensor_copy(out=raw, in_=psum_tiles[tt])
                pp = psKQ.tile([128, 512], f32, tag="proj", name=f"pp_{fidx}_{tt}")
                nc.tensor.matmul(pp, lhsT=perm_sb, rhs=raw, start=True, stop=True)
                t1 = work.tile([128, 512], bf16, tag="t1")
                nc.vector.tensor_tensor(t1, pp, ssb[:, ts], mult)
                t2 = work.tile([128, 512], bf16, tag="t2")
                nc.vector.tensor_tensor(t2, raw, cc[:, ts], mult)
                if copy_scaled:
                    nc.vector.tensor_tensor(qk_sb[:, fidx, ts], t2, t1, add)
                else:
                    nc.gpsimd.tensor_tensor(qk_sb[:, fidx, ts], t2, t1, add)

        # ---- K projection (both fc), chunk-paced off the xT DMA stream ----
        psK = {}
        for fc in range(2):
            for tt in range(4):
                psK[(fc, tt)] = psKQ.tile(
                    [128, 512], f32, tag="proj", name=f"k_{fc}_{tt}"
                )
        for kc in KORD:
            for fc in range(2):
                for tt in range(4):
                    nc.tensor.matmul(
                        psK[(fc, tt)],
                        lhsT=wk_sb[:, kc, fc * 128 : (fc + 1) * 128],
                        rhs=xT_sb[:, kc, tt * 512 : (tt + 1) * 512],
                        start=(kc == KORD[0]),
                        stop=(kc == KORD[-1]),
                    )
        for fc in range(2):
            proj_rope(2 + fc, [psK[(fc, tt)] for tt in range(4)], False)

        # ---- sum(x^2) ones-matmuls, then Q fc0 projection ----
        ss_sb = sbA.tile([1, T], f32, tag="ss")
        for s in range(4):
            ss_ps = psKQ.tile([1, 512], f32, tag="proj", name=f"ss_{s}")
            for kc in range(KC):
                nc.tensor.matmul(
                    ss_ps,
                    lhsT=ones_col,
                    rhs=xsq_sb[:, kc, s * 512 : (s + 1) * 512],
                    start=(kc == 0),
                    stop=(kc == KC - 1),
                )
            nc.scalar.copy(out=ss_sb[:, s * 512 : (s + 1) * 512], in_=ss_ps)
        psQ0 = [
            psKQ.tile([128, 512], f32, tag="proj", name=f"q0_{tt}")
            for tt in range(4)
        ]
        for kc in range(KC):
            for tt in range(4):
                nc.tensor.matmul(
                    psQ0[tt],
                    lhsT=wq_sb[:, kc, 0:128],
                    rhs=xT_sb[:, kc, tt * 512 : (tt + 1) * 512],
                    start=(kc == 0),
                    stop=(kc == KC - 1),
                )

        # r chain: ss -> sqrt(mean) -> 1/x -> r row; r_tok via PE transposes
        sq_sb = sbA.tile([1, T], f32, tag="sq")
        nc.scalar.activation(sq_sb, ss_sb, Sqrt, scale=1.0 / DIM)
        nc.vector.reciprocal(r_sb, sq_sb)
        rtok_ps = psKQ.tile([128, 16], f32, tag="proj", name="rtokps")
        for i in range(16):
            nc.tensor.transpose(
                rtok_ps[:, i : i + 1], r_sb[0:1, i * 128 : (i + 1) * 128],
                one_f32,
            )
        nc.vector.tensor_copy(out=r_tok, in_=rtok_ps)
        # r broadcast across partitions -> fold into Q-side rope tables
        r_bc = persist.tile([128, T], f32, tag="rbc")
        nc.gpsimd.partition_broadcast(r_bc, r_sb)
        nc.vector.tensor_tensor(cosr_sb, cos_sb, r_bc, mult)
        nc.vector.tensor_tensor(sinr_sb, sin_sb, r_bc, mult)

        proj_rope(0, psQ0, True)

        # ---- Q fc1 ----
        psQ1 = [
            psKQ.tile([128, 512], f32, tag="proj", name=f"q1_{tt}")
            for tt in range(4)
        ]
        for kc in range(KC):
            for tt in range(4):
                nc.tensor.matmul(
                    psQ1[tt],
                    lhsT=wq_sb[:, kc, 128:256],
                    rhs=xT_sb[:, kc, tt * 512 : (tt + 1) * 512],
                    start=(kc == 0),
                    stop=(kc == KC - 1),
                )
        proj_rope(1, psQ1, True)

        # ---- V projection (token-major) + r_tok scaling ----
        for tt in range(16):
            psv = psKQ.tile([128, 256], f32, tag="proj", name=f"v_{tt}")
            for kc in range(KC):
                nc.tensor.matmul(
                    psv,
                    lhsT=xT_sb[:, kc, tt * 128 : (tt + 1) * 128],
                    rhs=wv_sb[:, kc, :],
                    start=(kc == 0),
                    stop=(kc == KC - 1),
                )
            nc.vector.tensor_scalar(
                out=v_sb[:, tt, :, 0:64],
                in0=psv.rearrange("p (h d) -> p h d", h=HPC),
                scalar1=r_tok[:, tt : tt + 1],
                scalar2=None,
                op0=mult,
            )
        ctxA.close()

        # ---- attention: scores [k,q] -> paired exp -> transposed AV ----
        # PSUM: sc pair-tiles (2 banks) x2 + av4 x2 + outproj po x2 = 8 banks
        with (
            tc.tile_pool(name="psSC", bufs=2, space="PSUM") as psSC,
            tc.tile_pool(name="psAV", bufs=2, space="PSUM") as psAV,
            tc.tile_pool(name="psO", bufs=2, space="PSUM") as psO,
            tc.tile_pool(name="expp", bufs=4) as expp,
            tc.tile_pool(name="recp", bufs=4) as recp,
        ):
            for qt in range(4):
                q0 = qt * 512
                for pi in range(2):
                    # full-bank tiles: matmul start=True zeroes the whole 2KB
                    # bank, so only the FIRST write into each bank uses it
                    av4 = [
                        psAV.tile([128, 4, 128], f32, tag="av4",
                                  name=f"av_{qt}_{pi}_{x}")
                        for x in range(2)
                    ]
                    for kb in range(4 * qt + 4):
                        c0 = max(0, kb * 128 - q0)
                        sc = psSC.tile(
                            [128, 1024], f32, tag="sc", name=f"sc_{qt}_{pi}_{kb}"
                        )
                        for x in range(2):
                            rX = slice(x * 64, x * 64 + 64)
                            nc.tensor.matmul(
                                sc[:, x * 512 + c0 : x * 512 + 512],
                                lhsT=qk_sb[rX, 2 + pi, kb * 128 : (kb + 1) * 128],
                                rhs=qk_sb[rX, pi, q0 + c0 : q0 + 512],
                                start=True,
                                stop=True,
                            )
                        ex = expp.tile([128, 1024], bf16, tag="exp")
                        if c0 == 0:
                            nc.scalar.activation(
                                ex, sc, Exp, scale=r_tok[:, kb : kb + 1]
                            )
                        else:
                            for x in range(2):
                                nc.scalar.activation(
                                    ex[:, x * 512 + c0 : x * 512 + 512],
                                    sc[:, x * 512 + c0 : x * 512 + 512],
                                    Exp,
                                    scale=r_tok[:, kb : kb + 1],
                                )
                        if kb >= 4 * qt:  # diagonal block: causal mask
                            for x in range(2):
                                nc.vector.tensor_tensor(
                                    ex[:, x * 512 + c0 : x * 512 + c0 + 128],
                                    ex[:, x * 512 + c0 : x * 512 + c0 + 128],
                                    mask_sb,
                                    mult,
                                )
                        for qbl in range(4):
                            qb = 4 * qt + qbl
                            if kb > qb:
                                continue
                            for x in range(2):
                                nc.tensor.matmul(
                                    av4[x][:, qbl, 0:65],
                                    lhsT=ex[:, x * 512 + qbl * 128 : x * 512 + (qbl + 1) * 128],
                                    rhs=v_sb[:, kb, 2 * pi + x, :],
                                    start=(kb == 0 and qbl == 0),
                                    stop=(kb == qb),
                                    skip_group_check=True,
                                )
                    # normalize (rows 0..63 / row 64) into token-major av_tok
                    for x in range(2):
                        h = 2 * pi + x
                        rec4 = recp.tile([128, 4], f32, tag="rec")
                        nc.vector.reciprocal(rec4, av4[x][:, :, 64:65])
                        for qbl in range(4):
                            nc.vector.tensor_scalar(
                                out=av_tok[:, 4 * qt + qbl, h * 64 : (h + 1) * 64],
                                in0=av4[x][:, qbl, 0:64],
                                scalar1=rec4[:, qbl : qbl + 1],
                                scalar2=None,
                                op0=mult,
                            )
                # back to feature-major via DMA-engine transposes
                for tt in range(4 * qt, 4 * qt + 4):
                    nc.sync.dma_start_transpose(
                        av_all[:, :, tt * 128 : (tt + 1) * 128],
                        av_tok[:, tt, :],
                    )
                # out-projection for this token quarter
                for do in range(8):
                    po = psO.tile([128, 512], f32, tag="po", name=f"o_{qt}_{do}")
                    for fc in range(2):
                        nc.tensor.matmul(
                            po,
                            lhsT=wo_sb[:, fc, do * 128 : (do + 1) * 128],
                            rhs=av_all[:, fc, q0 : q0 + 512],
                            start=(fc == 0),
                            stop=(fc == 1),
                        )
                    sel = (qt + do) % 2
                    ob = work.tile([128, 512], bf16, tag="ob")
                    if sel == 0:
                        nc.scalar.copy(out=ob, in_=po)
                    else:
                        nc.vector.tensor_copy(out=ob, in_=po)
                    nc.sync.dma_start(
                        out.rearrange("(do p) t -> p do t", p=128)[:, do, q0 : q0 + 512],
                        ob,
                    )
            if tap == "rtok":
                nc.sync.dma_start(dbg[:, :], r_tok)
            elif tap == "qk":
                nc.sync.dma_start(dbg[:, :, :], qk_sb)
            elif tap == "v":
                nc.sync.dma_start(dbg[:, :, :, :], v_sb)
            elif tap == "avtok":
                nc.sync.dma_start(dbg[:, :, :], av_tok)
            elif tap == "avall":
                nc.sync.dma_start(dbg[:, :, :], av_all)
    nc.compile()
    return nc


def _host_inputs(x, norm_w, w_qkv, w_o, sin, cos):
    """Build the 8 per-core input maps (all bf16)."""
    n = T
    w_eff = np.asarray(w_qkv, np.float64) * np.asarray(norm_w, np.float64)[:, None]
    sin_n = np.asarray(sin, np.float32)[:n]  # [T, 64]
    cos_n = np.asarray(cos, np.float32)[:n]
    sign = np.concatenate([-np.ones(32, np.float32), np.ones(32, np.float32)])
    cos_tile = np.tile(cos_n.T, (2, 1))  # [128, T]
    sin_tile = np.tile((sin_n * sign[None, :]).T, (2, 1))  # [128, T]
    perm = np.zeros((128, 128), np.float32)
    for m in range(128):
        d = m % 64
        k = m + 32 if d < 32 else m - 32
        perm[k, m] = 1.0
    ident_np = np.eye(128, dtype=np.float32)
    ql = np.arange(128)[None, :]
    key = np.arange(128)[:, None]
    masks = (ql >= key).astype(np.float32)

    in_maps = []
    for c in range(8):
        b, g = c // 4, c % 4
        fs = slice(g * F, (g + 1) * F)
        in_maps.append(
            {
                "xT": np.ascontiguousarray(np.asarray(x, np.float32)[b].T).astype(BF16),
                "wq": (w_eff[:, 0:DIM][:, fs] * (DIM_HEAD ** -0.5)).astype(BF16),
                "wk": w_eff[:, DIM : 2 * DIM][:, fs].astype(BF16),
                "wv": w_eff[:, 2 * DIM : 3 * DIM][:, fs].astype(BF16),
                "wo": np.asarray(w_o, np.float32)[fs, :].astype(BF16),
                "cosT": cos_tile.astype(BF16),
                "sinT": sin_tile.astype(BF16),
                "perm": perm.astype(BF16),
                "masks": masks.astype(BF16),
                "ident": ident_np.astype(BF16),
            }
        )
    return in_maps


def kernel(x, norm_w, w_qkv, w_o, b_o, sin, cos):
    from concourse.bass_utils import run_bass_kernel_spmd

    if "nc" not in _NC_CACHE:
        _NC_CACHE["nc"] = _build_nc()
    nc = _NC_CACHE["nc"]
    in_maps = _host_inputs(x, norm_w, w_qkv, w_o, sin, cos)
    trace = bool(int(os.environ.get("KERNEL_TRACE", "0")))
    res = run_bass_kernel_spmd(nc, in_maps, core_ids=list(range(8)), trace=trace)
    if trace and res.exec_time_ns is not None:
        print(f"HW exec time: {res.exec_time_ns} ns")
    outs = [r["out"].astype(np.float32) for r in res.results]  # [1024, T] fm
    b_o = np.asarray(b_o, np.float32)
    full = np.empty((B, T, DIM), np.float32)
    for b in range(B):
        acc = outs[b * 4] + outs[b * 4 + 1] + outs[b * 4 + 2] + outs[b * 4 + 3]
        full[b] = acc.T + b_o[None, :]
    return full


# revision 30
# speedup vs baseline: 1.1125x; 1.1125x over previous
"""Trainium2 8-core kernel for RMSNorm -> QKV -> RoPE -> causal SDPA -> out-proj.

Sharding: core c = b*4 + g handles batch b (of 2) and heads 4g..4g+3 (of 16).
Each core computes a partial out-projection [dim, tokens]; the host sums the
4 head-group partials per batch (the tensor-parallel "unshard") and adds b_o.

Cost-model-driven layout (TimelineSim charges matmuls by OUTPUT FREE SIZE
only — contraction depth and output partitions are free):
  - scores per (head, kb): [key 128, q free] trimmed to the causal triangle.
  - AV runs TRANSPOSED: out [q 128, d 65] so each accumulation step costs 65
    rows instead of ~512; the ones column gives the softmax denominator.
  - The normalized token-major AV result is returned to feature-major layout
    with DMA-engine transposes (14ns/32x32 tile, off the compute engines).
  - exp for a head PAIR is fused into one Activation instruction (the two
    heads' score tiles sit in adjacent PSUM banks).
  - r = rsqrt(mean x^2) rides into Q via r-scaled RoPE tables, into scores
    via the per-key `scale` operand of exp, and into V via a per-partition
    tensor_scalar during the PSUM->SBUF copy. r_tok (token-major r) comes
    from 16 free PE transposes of the r row.
  - PE is kept continuously busy (the cost model halves PE speed after any
    idle gap until 3us of continuous execution): the K projection starts at
    xT chunk 2 so the DMA stream stays ahead of the PE stream.
"""

import os

import numpy as np
import ml_dtypes

BF16 = ml_dtypes.bfloat16

DIM = 1024
HEADS = 16
DIM_HEAD = 64
T = 2048  # tokens per batch
B = 2
HPC = 4  # heads per core
F = HPC * DIM_HEAD  # 256 per-core head width
KC = DIM // 128  # 8 contraction chunks
KORD = [2, 3, 4, 5, 6, 7, 0, 1]  # kc order: first matmul waits for chunk 2

_NC_CACHE = {}


def _build_nc():
    import concourse.bacc as bacc
    import concourse.mybir as mybir
    import concourse.tile as tile
    from contextlib import ExitStack

    f32 = mybir.dt.float32
    bf16 = mybir.dt.bfloat16
    nc = bacc.Bacc()

    xT = nc.declare_dram_parameter("xT", [DIM, T], bf16, isOutput=False)
    wq = nc.declare_dram_parameter("wq", [DIM, F], bf16, isOutput=False)
    wk = nc.declare_dram_parameter("wk", [DIM, F], bf16, isOutput=False)
    wv = nc.declare_dram_parameter("wv", [DIM, F], bf16, isOutput=False)
    wo = nc.declare_dram_parameter("wo", [F, DIM], bf16, isOutput=False)
    cosT = nc.declare_dram_parameter("cosT", [128, T], bf16, isOutput=False)
    sinT = nc.declare_dram_parameter("sinT", [128, T], bf16, isOutput=False)
    perm = nc.declare_dram_parameter("perm", [128, 128], bf16, isOutput=False)
    masks = nc.declare_dram_parameter("masks", [128, 128], bf16, isOutput=False)
    ident = nc.declare_dram_parameter("ident", [128, 128], bf16, isOutput=False)
    out = nc.declare_dram_parameter("out", [DIM, T], bf16, isOutput=True)
    tap = os.environ.get("KTAP", "")
    dbg = None
    if tap:
        _tap_shapes = {
            "rtok": ([128, 16], f32),
            "qk": ([128, 4, T], bf16),
            "v": ([128, 16, HPC, 65], bf16),
            "avtok": ([128, 16, F], bf16),
            "avall": ([128, 2, T], bf16),
        }
        shp, dt = _tap_shapes[tap]
        dbg = nc.declare_dram_parameter("dbg", shp, dt, isOutput=True)

    Exp = mybir.ActivationFunctionType.Exp
    Sqrt = mybir.ActivationFunctionType.Sqrt
    mult = mybir.AluOpType.mult
    add = mybir.AluOpType.add

    with ExitStack() as ctx:
        tc = ctx.enter_context(tile.TileContext(nc))
        consts = ctx.enter_context(tc.tile_pool(name="consts", bufs=1))
        persist = ctx.enter_context(tc.tile_pool(name="persist", bufs=1))
        work = ctx.enter_context(tc.tile_pool(name="work", bufs=4))
        vecs = ctx.enter_context(tc.tile_pool(name="vecs", bufs=1))

        # ---- constants / inputs ----
        wk_sb = consts.tile([128, KC, F], bf16, tag="wk")
        wq_sb = consts.tile([128, KC, F], bf16, tag="wq")
        wv_sb = consts.tile([128, KC, F], bf16, tag="wv")
        wo_sb = consts.tile([128, 2, DIM], bf16, tag="wo")
        cos_sb = consts.tile([128, T], bf16, tag="cos")
        sin_sb = consts.tile([128, T], bf16, tag="sin")
        perm_sb = consts.tile([128, 128], bf16, tag="perm")
        mask_sb = consts.tile([128, 128], bf16, tag="mask")
        id_sb = consts.tile([128, 128], bf16, tag="ident")
        ones_col = consts.tile([128, 1], bf16, tag="onesc")
        one_f32 = consts.tile([1, 1], f32, tag="onef")
        xT_sb = persist.tile([128, KC, T], bf16, tag="xT")
        xT_r = xT.rearrange("(kc p) t -> p kc t", p=128)
        # wk first (first PE consumer), then xT chunks in consumption order
        # with the other weights slotted behind the early chunks
        nc.sync.dma_start(wk_sb, wk.rearrange("(kc p) f -> p kc f", p=128))
        for kc in KORD[:4]:
            nc.sync.dma_start(xT_sb[:, kc], xT_r[:, kc])
        nc.sync.dma_start(perm_sb, perm[:, :])
        nc.sync.dma_start(cos_sb, cosT[:, :])
        nc.sync.dma_start(sin_sb, sinT[:, :])
        for kc in KORD[4:]:
            nc.sync.dma_start(xT_sb[:, kc], xT_r[:, kc])
        nc.sync.dma_start(wq_sb, wq.rearrange("(kc p) f -> p kc f", p=128))
        nc.sync.dma_start(wv_sb, wv.rearrange("(kc p) f -> p kc f", p=128))
        nc.sync.dma_start(mask_sb, masks[:, :])
        nc.sync.dma_start(id_sb, ident[:, :])
        nc.sync.dma_start(wo_sb, wo.rearrange("(fc p) d -> p fc d", p=128))
        nc.vector.memset(ones_col, 1.0)
        nc.vector.memset(one_f32, 1.0)

        # persistent activations
        qk_sb = persist.tile([128, 4, T], bf16, tag="qk")  # 0,1=q fc0/1; 2,3=k
        v_sb = persist.tile([128, 16, HPC, 65], bf16, tag="v")
        av_tok = persist.tile([128, 16, F], bf16, tag="avtok")
        av_all = persist.tile([128, 2, T], bf16, tag="av")
        r_sb = vecs.tile([1, T], f32, tag="r")
        r_tok = vecs.tile([128, 16], f32, tag="rtok")
        r_bc = persist.tile([128, T], f32, tag="rbc")
        nc.vector.memset(v_sb[:, :, :, 64:65], 1.0)

        ctxA = ExitStack()
        psKQ = ctxA.enter_context(tc.tile_pool(name="psKQ", bufs=8, space="PSUM"))
        sbA = ctxA.enter_context(tc.tile_pool(name="sbA", bufs=1))
        xsq_sb = sbA.tile([128, KC, T], bf16, tag="xsq")

        # x^2 per chunk (DVE, chases the xT DMAs)
        for kc in KORD:
            nc.vector.tensor_mul(xsq_sb[:, kc], xT_sb[:, kc], xT_sb[:, kc])

        def proj_rope(fidx, psum_tiles, is_q):
            """Finish a Q/K projection: PSUM->SBUF copy (r-scaled for Q),
            rotate-half perm matmul, rope multiply-adds into qk_sb[fidx]."""
            for tt in range(4):
                ts = slice(tt * 512, (tt + 1) * 512)
                raw = work.tile([128, 512], bf16, tag="raw")
                if is_q:
                    nc.vector.tensor_tensor(raw, psum_tiles[tt], r_bc[:, ts], mult)
                elif tt % 2 == 0:
                    nc.scalar.copy(out=raw, in_=psum_tiles[tt])
                else:
                    nc.vector.tensor_copy(out=raw, in_=psum_tiles[tt])
                pp = psKQ.tile([128, 512], f32, tag="proj", name=f"pp_{fidx}_{tt}")
                nc.tensor.matmul(pp, lhsT=perm_sb, rhs=raw, start=True, stop=True)
                t1 = work.tile([128, 512], bf16, tag="t1")
                nc.vector.tensor_tensor(t1, pp, sin_sb[:, ts], mult)
                t2 = work.tile([128, 512], bf16, tag="t2")
                nc.vector.tensor_tensor(t2, raw, cos_sb[:, ts], mult)
                if is_q:
                    nc.vector.tensor_tensor(qk_sb[:, fidx, ts], t2, t1, add)
                else:
                    nc.gpsimd.tensor_tensor(qk_sb[:, fidx, ts], t2, t1, add)

        # ---- K projection (both fc), chunk-paced off the xT DMA stream ----
        psK = {}
        for fc in range(2):
            for tt in range(4):
                psK[(fc, tt)] = psKQ.tile(
                    [128, 512], f32, tag="proj", name=f"k_{fc}_{tt}"
                )
        for kc in KORD:
            for fc in range(2):
                for tt in range(4):
                    nc.tensor.matmul(
                        psK[(fc, tt)],
                        lhsT=wk_sb[:, kc, fc * 128 : (fc + 1) * 128],
                        rhs=xT_sb[:, kc, tt * 512 : (tt + 1) * 512],
                        start=(kc == KORD[0]),
                        stop=(kc == KORD[-1]),
                    )
        for fc in range(2):
            proj_rope(2 + fc, [psK[(fc, tt)] for tt in range(4)], False)

        # ---- sum(x^2) ones-matmuls with the r-chain pipelined per slice ----
        ss_sb = sbA.tile([1, T], f32, tag="ss")
        sq_sb = sbA.tile([1, T], f32, tag="sq")
        rtok_ps = psKQ.tile([128, 16], f32, tag="proj", name="rtokps")
        for s in range(4):
            ts = slice(s * 512, (s + 1) * 512)
            ss_ps = psKQ.tile([1, 512], f32, tag="proj", name=f"ss_{s}")
            for kc in range(KC):
                nc.tensor.matmul(
                    ss_ps,
                    lhsT=ones_col,
                    rhs=xsq_sb[:, kc, s * 512 : (s + 1) * 512],
                    start=(kc == 0),
                    stop=(kc == KC - 1),
                )
            nc.scalar.copy(out=ss_sb[:, ts], in_=ss_ps)
            nc.scalar.activation(sq_sb[:, ts], ss_sb[:, ts], Sqrt, scale=1.0 / DIM)
            nc.vector.reciprocal(r_sb[:, ts], sq_sb[:, ts])
            nc.gpsimd.partition_broadcast(r_bc[:, ts], r_sb[:, ts])
        # Q fc0 projection
        psQ0 = [
            psKQ.tile([128, 512], f32, tag="proj", name=f"q0_{tt}")
            for tt in range(4)
        ]
        for kc in range(KC):
            for tt in range(4):
                nc.tensor.matmul(
                    psQ0[tt],
                    lhsT=wq_sb[:, kc, 0:128],
                    rhs=xT_sb[:, kc, tt * 512 : (tt + 1) * 512],
                    start=(kc == 0),
                    stop=(kc == KC - 1),
                )
        # r_tok via PE transposes of the r row
        for i in range(16):
            nc.tensor.transpose(
                rtok_ps[:, i : i + 1], r_sb[0:1, i * 128 : (i + 1) * 128],
                one_f32,
            )
        nc.vector.tensor_copy(out=r_tok, in_=rtok_ps)

        proj_rope(0, psQ0, True)

        # ---- Q fc1 ----
        psQ1 = [
            psKQ.tile([128, 512], f32, tag="proj", name=f"q1_{tt}")
            for tt in range(4)
        ]
        for kc in range(KC):
            for tt in range(4):
                nc.tensor.matmul(
                    psQ1[tt],
                    lhsT=wq_sb[:, kc, 128:256],
                    rhs=xT_sb[:, kc, tt * 512 : (tt + 1) * 512],
                    start=(kc == 0),
                    stop=(kc == KC - 1),
                )
        proj_rope(1, psQ1, True)

        # ---- V projection (token-major) + r_tok scaling ----
        ctxA.close()

        # ---- attention: scores [k,q] -> paired exp -> transposed AV ----
        # PSUM: sc ring (2x2 banks, also V-proj) + av4/po ring (3 banks) = 7
        with (
            tc.tile_pool(name="psSC", bufs=2, space="PSUM") as psSC,
            tc.tile_pool(name="psAV", bufs=3, space="PSUM") as psAV,
            tc.tile_pool(name="expp", bufs=4) as expp,
            tc.tile_pool(name="recp", bufs=4) as recp,
        ):
            for qt in range(4):
                q0 = qt * 512
                # V projection for this quarter's key blocks
                for tt in range(4 * qt, 4 * qt + 4):
                    psv = psSC.tile([128, 256], f32, tag="sc", name=f"v_{tt}")
                    for kc in range(KC):
                        nc.tensor.matmul(
                            psv,
                            lhsT=xT_sb[:, kc, tt * 128 : (tt + 1) * 128],
                            rhs=wv_sb[:, kc, :],
                            start=(kc == 0),
                            stop=(kc == KC - 1),
                        )
                    nc.vector.tensor_scalar(
                        out=v_sb[:, tt, :, 0:64],
                        in0=psv.rearrange("p (h d) -> p h d", h=HPC),
                        scalar1=r_tok[:, tt : tt + 1],
                        scalar2=None,
                        op0=mult,
                    )
                for pi in range(2):
                    # full-bank tiles: matmul start=True zeroes the whole 2KB
                    # bank, so only the FIRST write into each bank uses it
                    av4 = [
                        psAV.tile([128, 4, 128], f32, tag="av4",
                                  name=f"av_{qt}_{pi}_{x}")
                        for x in range(2)
                    ]
                    for kb in range(4 * qt + 4):
                        c0 = max(0, kb * 128 - q0)
                        sc = psSC.tile(
                            [128, 1024], f32, tag="sc", name=f"sc_{qt}_{pi}_{kb}"
                        )
                        for x in range(2):
                            rX = slice(x * 64, x * 64 + 64)
                            nc.tensor.matmul(
                                sc[:, x * 512 + c0 : x * 512 + 512],
                                lhsT=qk_sb[rX, 2 + pi, kb * 128 : (kb + 1) * 128],
                                rhs=qk_sb[rX, pi, q0 + c0 : q0 + 512],
                                start=True,
                                stop=True,
                            )
                        ex = expp.tile([128, 1024], bf16, tag="exp")
                        if c0 == 0:
                            nc.scalar.activation(
                                ex, sc, Exp, scale=r_tok[:, kb : kb + 1]
                            )
                        else:
                            for x in range(2):
                                nc.scalar.activation(
                                    ex[:, x * 512 + c0 : x * 512 + 512],
                                    sc[:, x * 512 + c0 : x * 512 + 512],
                                    Exp,
                                    scale=r_tok[:, kb : kb + 1],
                                )
                        if kb >= 4 * qt:  # diagonal block: causal mask
                            for x in range(2):
                                nc.gpsimd.tensor_tensor(
                                    ex[:, x * 512 + c0 : x * 512 + c0 + 128],
                                    ex[:, x * 512 + c0 : x * 512 + c0 + 128],
                                    mask_sb,
                                    mult,
                                )
                        for qbl in range(4):
                            qb = 4 * qt + qbl
                            if kb > qb:
                                continue
                            for x in range(2):
                                nc.tensor.matmul(
                                    av4[x][:, qbl, 0:65],
                                    lhsT=ex[:, x * 512 + qbl * 128 : x * 512 + (qbl + 1) * 128],
                                    rhs=v_sb[:, kb, 2 * pi + x, :],
                                    start=(kb == 0 and qbl == 0),
                                    stop=(kb == qb),
                                    skip_group_check=True,
                                )
                    # normalize (rows 0..63 / row 64) into token-major av_tok
                    for x in range(2):
                        h = 2 * pi + x
                        rec4 = recp.tile([128, 4], f32, tag="rec")
                        nc.vector.reciprocal(rec4, av4[x][:, :, 64:65])
                        for qbl in range(4):
                            nc.vector.tensor_scalar(
                                out=av_tok[:, 4 * qt + qbl, h * 64 : (h + 1) * 64],
                                in0=av4[x][:, qbl, 0:64],
                                scalar1=rec4[:, qbl : qbl + 1],
                                scalar2=None,
                                op0=mult,
                            )
                # back to feature-major via DMA-engine transposes
                for tt in range(4 * qt, 4 * qt + 4):
                    nc.sync.dma_start_transpose(
                        av_all[:, :, tt * 128 : (tt + 1) * 128],
                        av_tok[:, tt, :],
                    )
                # out-projection for this token quarter
                for do in range(8):
                    po = psAV.tile([128, 512], f32, tag="av4", name=f"o_{qt}_{do}")
                    for fc in range(2):
                        nc.tensor.matmul(
                            po,
                            lhsT=wo_sb[:, fc, do * 128 : (do + 1) * 128],
                            rhs=av_all[:, fc, q0 : q0 + 512],
                            start=(fc == 0),
                            stop=(fc == 1),
                        )
                    ob = work.tile([128, 512], bf16, tag="ob")
                    nc.vector.tensor_copy(out=ob, in_=po)
                    nc.sync.dma_start(
                        out.rearrange("(do p) t -> p do t", p=128)[:, do, q0 : q0 + 512],
                        ob,
                    )
            if tap == "rtok":
                nc.sync.dma_start(dbg[:, :], r_tok)
            elif tap == "qk":
                nc.sync.dma_start(dbg[:, :, :], qk_sb)
            elif tap == "v":
                nc.sync.dma_start(dbg[:, :, :, :], v_sb)
            elif tap == "avtok":
                nc.sync.dma_start(dbg[:, :, :], av_tok)
            elif tap == "avall":
                nc.sync.dma_start(dbg[:, :, :], av_all)
    nc.compile()
    return nc


def _host_inputs(x, norm_w, w_qkv, w_o, sin, cos):
    """Build the 8 per-core input maps (all bf16)."""
    n = T
    w_eff = np.asarray(w_qkv, np.float64) * np.asarray(norm_w, np.float64)[:, None]
    sin_n = np.asarray(sin, np.float32)[:n]  # [T, 64]
    cos_n = np.asarray(cos, np.float32)[:n]
    sign = np.concatenate([-np.ones(32, np.float32), np.ones(32, np.float32)])
    cos_tile = np.tile(cos_n.T, (2, 1))  # [128, T]
    sin_tile = np.tile((sin_n * sign[None, :]).T, (2, 1))  # [128, T]
    perm = np.zeros((128, 128), np.float32)
    for m in range(128):
        d = m % 64
        k = m + 32 if d < 32 else m - 32
        perm[k, m] = 1.0
    ident_np = np.eye(128, dtype=np.float32)
    ql = np.arange(128)[None, :]
    key = np.arange(128)[:, None]
    masks = (ql >= key).astype(np.float32)

    in_maps = []
    for c in range(8):
        b, g = c // 4, c % 4
        fs = slice(g * F, (g + 1) * F)
        in_maps.append(
            {
                "xT": np.ascontiguousarray(np.asarray(x, np.float32)[b].T).astype(BF16),
                "wq": (w_eff[:, 0:DIM][:, fs] * (DIM_HEAD ** -0.5)).astype(BF16),
                "wk": w_eff[:, DIM : 2 * DIM][:, fs].astype(BF16),
                "wv": w_eff[:, 2 * DIM : 3 * DIM][:, fs].astype(BF16),
                "wo": np.asarray(w_o, np.float32)[fs, :].astype(BF16),
                "cosT": cos_tile.astype(BF16),
                "sinT": sin_tile.astype(BF16),
                "perm": perm.astype(BF16),
                "masks": masks.astype(BF16),
                "ident": ident_np.astype(BF16),
            }
        )
    return in_maps


def kernel(x, norm_w, w_qkv, w_o, b_o, sin, cos):
    from concourse.bass_utils import run_bass_kernel_spmd

    if "nc" not in _NC_CACHE:
        _NC_CACHE["nc"] = _build_nc()
    nc = _NC_CACHE["nc"]
    in_maps = _host_inputs(x, norm_w, w_qkv, w_o, sin, cos)
    trace = bool(int(os.environ.get("KERNEL_TRACE", "0")))
    res = run_bass_kernel_spmd(nc, in_maps, core_ids=list(range(8)), trace=trace)
    if trace and res.exec_time_ns is not None:
        print(f"HW exec time: {res.exec_time_ns} ns")
    outs = [r["out"].astype(np.float32) for r in res.results]  # [1024, T] fm
    b_o = np.asarray(b_o, np.float32)
    full = np.empty((B, T, DIM), np.float32)
    for b in range(B):
        acc = outs[b * 4] + outs[b * 4 + 1] + outs[b * 4 + 2] + outs[b * 4 + 3]
        full[b] = acc.T + b_o[None, :]
    return full


# revision 37
# speedup vs baseline: 1.1856x; 1.0657x over previous
"""Trainium2 8-core kernel for RMSNorm -> QKV -> RoPE -> causal SDPA -> out-proj.

Sharding: core c = b*4 + g handles batch b (of 2) and heads 4g..4g+3 (of 16).
Each core computes a partial out-projection [dim, tokens]; the host sums the
4 head-group partials per batch (the tensor-parallel "unshard") and adds b_o.

Cost-model-driven layout (TimelineSim charges matmuls by OUTPUT FREE SIZE
only — contraction depth and output partitions are free):
  - scores per (head, kb): [key 128, q free] trimmed to the causal triangle.
  - AV runs TRANSPOSED: out [q 128, d 65] so each accumulation step costs 65
    rows instead of ~512; the ones column gives the softmax denominator.
  - The normalized token-major AV result is returned to feature-major layout
    with DMA-engine transposes (14ns/32x32 tile, off the compute engines).
  - exp for a head PAIR is fused into one Activation instruction (the two
    heads' score tiles sit in adjacent PSUM banks).
  - r = rsqrt(mean x^2) rides into Q via r-scaled RoPE tables, into scores
    via the per-key `scale` operand of exp, and into V via a per-partition
    tensor_scalar during the PSUM->SBUF copy. r_tok (token-major r) comes
    from 16 free PE transposes of the r row.
  - PE is kept continuously busy (the cost model halves PE speed after any
    idle gap until 3us of continuous execution): the K projection starts at
    xT chunk 2 so the DMA stream stays ahead of the PE stream.
"""

import os

import numpy as np
import ml_dtypes

BF16 = ml_dtypes.bfloat16

DIM = 1024
HEADS = 16
DIM_HEAD = 64
T = 2048  # tokens per batch
B = 2
HPC = 4  # heads per core
F = HPC * DIM_HEAD  # 256 per-core head width
KC = DIM // 128  # 8 contraction chunks
KORD = [2, 3, 4, 5, 6, 7, 0, 1]  # kc order: first matmul waits for chunk 2

_NC_CACHE = {}


def _build_nc():
    import concourse.bacc as bacc
    import concourse.mybir as mybir
    import concourse.tile as tile
    from contextlib import ExitStack

    f32 = mybir.dt.float32
    bf16 = mybir.dt.bfloat16
    nc = bacc.Bacc()

    xT = nc.declare_dram_parameter("xT", [DIM, T], bf16, isOutput=False)
    wq = nc.declare_dram_parameter("wq", [DIM, F], bf16, isOutput=False)
    wk = nc.declare_dram_parameter("wk", [DIM, F], bf16, isOutput=False)
    wv = nc.declare_dram_parameter("wv", [DIM, F], bf16, isOutput=False)
    wo = nc.declare_dram_parameter("wo", [F, DIM], bf16, isOutput=False)
    cosT = nc.declare_dram_parameter("cosT", [128, T], bf16, isOutput=False)
    sinT = nc.declare_dram_parameter("sinT", [128, T], bf16, isOutput=False)
    perm = nc.declare_dram_parameter("perm", [128, 128], bf16, isOutput=False)
    masks = nc.declare_dram_parameter("masks", [128, 128], bf16, isOutput=False)
    ident = nc.declare_dram_parameter("ident", [128, 128], bf16, isOutput=False)
    out = nc.declare_dram_parameter("out", [DIM, T], bf16, isOutput=True)
    tap = os.environ.get("KTAP", "")
    dbg = None
    if tap:
        _tap_shapes = {
            "rtok": ([128, 16], f32),
            "qk": ([128, 4, T], bf16),
            "v": ([128, 16, HPC, 65], bf16),
            "avtok": ([128, 16, F], bf16),
            "avall": ([128, 2, T], bf16),
        }
        shp, dt = _tap_shapes[tap]
        dbg = nc.declare_dram_parameter("dbg", shp, dt, isOutput=True)

    Exp = mybir.ActivationFunctionType.Exp
    Sqrt = mybir.ActivationFunctionType.Sqrt
    mult = mybir.AluOpType.mult
    add = mybir.AluOpType.add

    with ExitStack() as ctx:
        tc = ctx.enter_context(tile.TileContext(nc))
        consts = ctx.enter_context(tc.tile_pool(name="consts", bufs=1))
        persist = ctx.enter_context(tc.tile_pool(name="persist", bufs=1))
        work = ctx.enter_context(tc.tile_pool(name="work", bufs=4))
        vecs = ctx.enter_context(tc.tile_pool(name="vecs", bufs=1))

        # ---- constants / inputs ----
        wk_sb = consts.tile([128, KC, F], bf16, tag="wk")
        wq_sb = consts.tile([128, KC, F], bf16, tag="wq")
        wv_sb = consts.tile([128, KC, F], bf16, tag="wv")
        wo_sb = consts.tile([128, 2, DIM], bf16, tag="wo")
        cos_sb = consts.tile([128, T], bf16, tag="cos")
        sin_sb = consts.tile([128, T], bf16, tag="sin")
        perm_sb = consts.tile([128, 128], bf16, tag="perm")
        mask_sb = consts.tile([128, 128], bf16, tag="mask")
        id_sb = consts.tile([128, 128], bf16, tag="ident")
        ones_col = consts.tile([128, 1], bf16, tag="onesc")
        one_f32 = consts.tile([1, 1], f32, tag="onef")
        xT_sb = persist.tile([128, KC, T], bf16, tag="xT")
        xT_r = xT.rearrange("(kc p) t -> p kc t", p=128)
        # wk first (first PE consumer), then xT chunks in consumption order
        # with the other weights slotted behind the early chunks
        nc.sync.dma_start(wk_sb, wk.rearrange("(kc p) f -> p kc f", p=128))
        for kc in KORD[:4]:
            nc.sync.dma_start(xT_sb[:, kc], xT_r[:, kc])
        nc.sync.dma_start(perm_sb, perm[:, :])
        nc.sync.dma_start(cos_sb, cosT[:, :])
        nc.sync.dma_start(sin_sb, sinT[:, :])
        for kc in KORD[4:]:
            nc.sync.dma_start(xT_sb[:, kc], xT_r[:, kc])
        nc.sync.dma_start(wq_sb, wq.rearrange("(kc p) f -> p kc f", p=128))
        nc.sync.dma_start(wv_sb, wv.rearrange("(kc p) f -> p kc f", p=128))
        nc.sync.dma_start(mask_sb, masks[:, :])
        nc.sync.dma_start(id_sb, ident[:, :])
        nc.sync.dma_start(wo_sb, wo.rearrange("(fc p) d -> p fc d", p=128))
        nc.vector.memset(ones_col, 1.0)
        nc.vector.memset(one_f32, 1.0)

        # persistent activations
        qk_sb = persist.tile([128, 4, T], bf16, tag="qk")  # 0,1=q fc0/1; 2,3=k
        v_sb = persist.tile([128, 16, HPC, 65], bf16, tag="v")
        av_tok = persist.tile([128, 16, F], bf16, tag="avtok")
        av_all = persist.tile([128, 2, T], bf16, tag="av")
        r_sb = vecs.tile([1, T], f32, tag="r")
        r_tok = vecs.tile([128, 16], f32, tag="rtok")
        r_bc = persist.tile([128, T], f32, tag="rbc")
        cosr_sb = persist.tile([128, T], bf16, tag="cosr")
        sinr_sb = persist.tile([128, T], bf16, tag="sinr")
        nc.vector.memset(v_sb[:, :, :, 64:65], 1.0)

        ctxA = ExitStack()
        psKQ = ctxA.enter_context(tc.tile_pool(name="psKQ", bufs=8, space="PSUM"))
        sbA = ctxA.enter_context(tc.tile_pool(name="sbA", bufs=1))
        xsq_sb = sbA.tile([128, KC, T], bf16, tag="xsq")

        # x^2 per chunk (DVE, chases the xT DMAs)
        for kc in KORD:
            nc.vector.tensor_mul(xsq_sb[:, kc], xT_sb[:, kc], xT_sb[:, kc])

        def proj_rope(fidx, psum_tiles, is_q):
            """Finish a Q/K projection: PSUM->SBUF copy, rotate-half perm
            matmul, rope multiply-adds into qk_sb[fidx]. Q uses the r-scaled
            tables so r_q rides in for free."""
            cc = cosr_sb if is_q else cos_sb
            ssb = sinr_sb if is_q else sin_sb
            for tt in range(4):
                ts = slice(tt * 512, (tt + 1) * 512)
                raw = work.tile([128, 512], bf16, tag="raw")
                if is_q or tt % 2 == 0:
                    nc.scalar.copy(out=raw, in_=psum_tiles[tt])
                else:
                    nc.vector.tensor_copy(out=raw, in_=psum_tiles[tt])
                pp = psKQ.tile([128, 512], f32, tag="proj", name=f"pp_{fidx}_{tt}")
                nc.tensor.matmul(pp, lhsT=perm_sb, rhs=raw, start=True, stop=True)
                t1 = work.tile([128, 512], bf16, tag="t1")
                nc.vector.tensor_tensor(t1, pp, ssb[:, ts], mult)
                t2 = work.tile([128, 512], bf16, tag="t2")
                nc.vector.tensor_tensor(t2, raw, cc[:, ts], mult)
                if is_q:
                    nc.vector.tensor_tensor(qk_sb[:, fidx, ts], t2, t1, add)
                else:
                    nc.gpsimd.tensor_tensor(qk_sb[:, fidx, ts], t2, t1, add)

        # ---- K projection (both fc), chunk-paced off the xT DMA stream ----
        psK = {}
        for fc in range(2):
            for tt in range(4):
                psK[(fc, tt)] = psKQ.tile(
                    [128, 512], f32, tag="proj", name=f"k_{fc}_{tt}"
                )
        for kc in KORD:
            for fc in range(2):
                for tt in range(4):
                    nc.tensor.matmul(
                        psK[(fc, tt)],
                        lhsT=wk_sb[:, kc, fc * 128 : (fc + 1) * 128],
                        rhs=xT_sb[:, kc, tt * 512 : (tt + 1) * 512],
                        start=(kc == KORD[0]),
                        stop=(kc == KORD[-1]),
                    )
        for fc in range(2):
            proj_rope(2 + fc, [psK[(fc, tt)] for tt in range(4)], False)

        # ---- sum(x^2) ones-matmuls with the r-chain pipelined per slice ----
        ss_sb = sbA.tile([1, T], f32, tag="ss")
        sq_sb = sbA.tile([1, T], f32, tag="sq")
        for s in range(4):
            ts = slice(s * 512, (s + 1) * 512)
            ss_ps = psKQ.tile([1, 512], f32, tag="proj", name=f"ss_{s}")
            for kc in range(KC):
                nc.tensor.matmul(
                    ss_ps,
                    lhsT=ones_col,
                    rhs=xsq_sb[:, kc, s * 512 : (s + 1) * 512],
                    start=(kc == 0),
                    stop=(kc == KC - 1),
                )
            nc.scalar.copy(out=ss_sb[:, ts], in_=ss_ps)
            nc.scalar.activation(sq_sb[:, ts], ss_sb[:, ts], Sqrt, scale=1.0 / DIM)
            nc.vector.reciprocal(r_sb[:, ts], sq_sb[:, ts])
            nc.gpsimd.partition_broadcast(r_bc[:, ts], r_sb[:, ts])
            nc.gpsimd.tensor_tensor(cosr_sb[:, ts], cos_sb[:, ts], r_bc[:, ts], mult)
            nc.gpsimd.tensor_tensor(sinr_sb[:, ts], sin_sb[:, ts], r_bc[:, ts], mult)
        # Q fc0 projection
        psQ0 = [
            psKQ.tile([128, 512], f32, tag="proj", name=f"q0_{tt}")
            for tt in range(4)
        ]
        for kc in range(KC):
            for tt in range(4):
                nc.tensor.matmul(
                    psQ0[tt],
                    lhsT=wq_sb[:, kc, 0:128],
                    rhs=xT_sb[:, kc, tt * 512 : (tt + 1) * 512],
                    start=(kc == 0),
                    stop=(kc == KC - 1),
                )
        # r_tok via PE transposes of the r row
        rtok_ps = psKQ.tile([128, 16], f32, tag="proj", name="rtokps")
        for i in range(16):
            nc.tensor.transpose(
                rtok_ps[:, i : i + 1], r_sb[0:1, i * 128 : (i + 1) * 128],
                one_f32,
            )
        nc.vector.tensor_copy(out=r_tok, in_=rtok_ps)

        proj_rope(0, psQ0, True)

        # ---- Q fc1 ----
        psQ1 = [
            psKQ.tile([128, 512], f32, tag="proj", name=f"q1_{tt}")
            for tt in range(4)
        ]
        for kc in range(KC):
            for tt in range(4):
                nc.tensor.matmul(
                    psQ1[tt],
                    lhsT=wq_sb[:, kc, 128:256],
                    rhs=xT_sb[:, kc, tt * 512 : (tt + 1) * 512],
                    start=(kc == 0),
                    stop=(kc == KC - 1),
                )
        proj_rope(1, psQ1, True)

        # ---- V projection (token-major) + r_tok scaling ----
        ctxA.close()

        # ---- attention: scores [k,q] -> paired exp -> transposed AV ----
        # PSUM: sc ring (2x2 banks, also V-proj) + av4/po ring (3) + avT (1)
        with (
            tc.tile_pool(name="psSC", bufs=2, space="PSUM") as psSC,
            tc.tile_pool(name="psAV", bufs=3, space="PSUM") as psAV,
            tc.tile_pool(name="psT", bufs=1, space="PSUM") as psT,
            tc.tile_pool(name="expp", bufs=4) as expp,
            tc.tile_pool(name="recp", bufs=4) as recp,
        ):
            def v_proj(tt):
                psv = psSC.tile([128, 256], f32, tag="sc", name=f"v_{tt}")
                for kc in range(KC):
                    nc.tensor.matmul(
                        psv,
                        lhsT=xT_sb[:, kc, tt * 128 : (tt + 1) * 128],
                        rhs=wv_sb[:, kc, :],
                        start=(kc == 0),
                        stop=(kc == KC - 1),
                    )
                nc.vector.tensor_scalar(
                    out=v_sb[:, tt, :, 0:64],
                    in0=psv.rearrange("p (h d) -> p h d", h=HPC),
                    scalar1=r_tok[:, tt : tt + 1],
                    scalar2=None,
                    op0=mult,
                )

            for qt in range(4):
                q0 = qt * 512
                for pi in range(2):
                    # full-bank tiles: matmul start=True zeroes the whole 2KB
                    # bank, so only the FIRST write into each bank uses it
                    av4 = [
                        psAV.tile([128, 4, 128], f32, tag="av4",
                                  name=f"av_{qt}_{pi}_{x}")
                        for x in range(2)
                    ]
                    for kb in range(4 * qt + 4):
                        # stagger this quarter's V projections into pair 0's
                        # kb loop so PE fills the gaps while Act runs exp
                        if pi == 0 and qt == 0:
                            v_proj(kb)
                        c0 = max(0, kb * 128 - q0)
                        sc = psSC.tile(
                            [128, 1024], f32, tag="sc", name=f"sc_{qt}_{pi}_{kb}"
                        )
                        for x in range(2):
                            rX = slice(x * 64, x * 64 + 64)
                            nc.tensor.matmul(
                                sc[:, x * 512 + c0 : x * 512 + 512],
                                lhsT=qk_sb[rX, 2 + pi, kb * 128 : (kb + 1) * 128],
                                rhs=qk_sb[rX, pi, q0 + c0 : q0 + 512],
                                start=True,
                                stop=True,
                            )
                        ex = expp.tile([128, 1024], bf16, tag="exp")
                        if c0 == 0:
                            nc.scalar.activation(
                                ex, sc, Exp, scale=r_tok[:, kb : kb + 1]
                            )
                        else:
                            for x in range(2):
                                nc.scalar.activation(
                                    ex[:, x * 512 + c0 : x * 512 + 512],
                                    sc[:, x * 512 + c0 : x * 512 + 512],
                                    Exp,
                                    scale=r_tok[:, kb : kb + 1],
                                )
                        if kb >= 4 * qt:  # diagonal block: causal mask
                            for x in range(2):
                                nc.gpsimd.tensor_tensor(
                                    ex[:, x * 512 + c0 : x * 512 + c0 + 128],
                                    ex[:, x * 512 + c0 : x * 512 + c0 + 128],
                                    mask_sb,
                                    mult,
                                )
                        for qbl in range(4):
                            qb = 4 * qt + qbl
                            if kb > qb:
                                continue
                            for x in range(2):
                                nc.tensor.matmul(
                                    av4[x][:, qbl, 0:65],
                                    lhsT=ex[:, x * 512 + qbl * 128 : x * 512 + (qbl + 1) * 128],
                                    rhs=v_sb[:, kb, 2 * pi + x, :],
                                    start=(kb == 0 and qbl == 0),
                                    stop=(kb == qb),
                                    skip_group_check=True,
                                )
                        if pi == 0 and qt > 0 and kb < 4:
                            v_proj(4 * qt + kb)
                    # normalize (rows 0..63 / row 64) into token-major av_tok
                    for x in range(2):
                        h = 2 * pi + x
                        rec4 = recp.tile([128, 4], f32, tag="rec")
                        nc.vector.reciprocal(rec4, av4[x][:, :, 64:65])
                        for qbl in range(4):
                            nc.vector.tensor_scalar(
                                out=av_tok[:, 4 * qt + qbl, h * 64 : (h + 1) * 64],
                                in0=av4[x][:, qbl, 0:64],
                                scalar1=rec4[:, qbl : qbl + 1],
                                scalar2=None,
                                op0=mult,
                            )
                # back to feature-major via PE transposes (53ns each)
                avT = psT.tile([128, 8, 128], bf16, tag="avt", name=f"avt_{qt}")
                for j, tt in enumerate(range(4 * qt, 4 * qt + 4)):
                    for fc in range(2):
                        nc.tensor.transpose(
                            avT[:, fc * 4 + j, :],
                            av_tok[:, tt, fc * 128 : (fc + 1) * 128],
                            id_sb,
                        )
                for fc in range(2):
                    nc.vector.tensor_copy(
                        out=av_all[:, fc, q0 : q0 + 512],
                        in_=avT[:, fc * 4 : fc * 4 + 4, :],
                    )
                # out-projection for this token quarter
                for do in range(8):
                    po = psAV.tile([128, 512], f32, tag="av4", name=f"o_{qt}_{do}")
                    for fc in range(2):
                        nc.tensor.matmul(
                            po,
                            lhsT=wo_sb[:, fc, do * 128 : (do + 1) * 128],
                            rhs=av_all[:, fc, q0 : q0 + 512],
                            start=(fc == 0),
                            stop=(fc == 1),
                        )
                    ob = work.tile([128, 512], bf16, tag="ob")
                    nc.vector.tensor_copy(out=ob, in_=po)
                    nc.sync.dma_start(
                        out.rearrange("(do p) t -> p do t", p=128)[:, do, q0 : q0 + 512],
                        ob,
                    )
            if tap == "rtok":
                nc.sync.dma_start(dbg[:, :], r_tok)
            elif tap == "qk":
                nc.sync.dma_start(dbg[:, :, :], qk_sb)
            elif tap == "v":
                nc.sync.dma_start(dbg[:, :, :, :], v_sb)
            elif tap == "avtok":
                nc.sync.dma_start(dbg[:, :, :], av_tok)
            elif tap == "avall":
                nc.sync.dma_start(dbg[:, :, :], av_all)
    nc.compile()
    return nc


def _host_inputs(x, norm_w, w_qkv, w_o, sin, cos):
    """Build the 8 per-core input maps (all bf16)."""
    n = T
    w_eff = np.asarray(w_qkv, np.float64) * np.asarray(norm_w, np.float64)[:, None]
    sin_n = np.asarray(sin, np.float32)[:n]  # [T, 64]
    cos_n = np.asarray(cos, np.float32)[:n]
    sign = np.concatenate([-np.ones(32, np.float32), np.ones(32, np.float32)])
    cos_tile = np.tile(cos_n.T, (2, 1))  # [128, T]
    sin_tile = np.tile((sin_n * sign[None, :]).T, (2, 1))  # [128, T]
    perm = np.zeros((128, 128), np.float32)
    for m in range(128):
        d = m % 64
        k = m + 32 if d < 32 else m - 32
        perm[k, m] = 1.0
    ident_np = np.eye(128, dtype=np.float32)
    ql = np.arange(128)[None, :]
    key = np.arange(128)[:, None]
    masks = (ql >= key).astype(np.float32)

    in_maps = []
    for c in range(8):
        b, g = c // 4, c % 4
        fs = slice(g * F, (g + 1) * F)
        in_maps.append(
            {
                "xT": np.ascontiguousarray(np.asarray(x, np.float32)[b].T).astype(BF16),
                "wq": (w_eff[:, 0:DIM][:, fs] * (DIM_HEAD ** -0.5)).astype(BF16),
                "wk": w_eff[:, DIM : 2 * DIM][:, fs].astype(BF16),
                "wv": w_eff[:, 2 * DIM : 3 * DIM][:, fs].astype(BF16),
                "wo": np.asarray(w_o, np.float32)[fs, :].astype(BF16),
                "cosT": cos_tile.astype(BF16),
                "sinT": sin_tile.astype(BF16),
                "perm": perm.astype(BF16),
                "masks": masks.astype(BF16),
                "ident": ident_np.astype(BF16),
            }
        )
    return in_maps


def kernel(x, norm_w, w_qkv, w_o, b_o, sin, cos):
    from concourse.bass_utils import run_bass_kernel_spmd

    if "nc" not in _NC_CACHE:
        _NC_CACHE["nc"] = _build_nc()
    nc = _NC_CACHE["nc"]
    in_maps = _host_inputs(x, norm_w, w_qkv, w_o, sin, cos)
    trace = bool(int(os.environ.get("KERNEL_TRACE", "0")))
    res = run_bass_kernel_spmd(nc, in_maps, core_ids=list(range(8)), trace=trace)
    if trace and res.exec_time_ns is not None:
        print(f"HW exec time: {res.exec_time_ns} ns")
    outs = [r["out"].astype(np.float32) for r in res.results]  # [1024, T] fm
    b_o = np.asarray(b_o, np.float32)
    full = np.empty((B, T, DIM), np.float32)
    for b in range(B):
        acc = outs[b * 4] + outs[b * 4 + 1] + outs[b * 4 + 2] + outs[b * 4 + 3]
        full[b] = acc.T + b_o[None, :]
    return full


# revision 38
# speedup vs baseline: 1.1926x; 1.0058x over previous
"""Trainium2 8-core kernel for RMSNorm -> QKV -> RoPE -> causal SDPA -> out-proj.

Sharding: core c = b*4 + g handles batch b (of 2) and heads 4g..4g+3 (of 16).
Each core computes a partial out-projection [dim, tokens]; the host sums the
4 head-group partials per batch (the tensor-parallel "unshard") and adds b_o.

Cost-model-driven layout (TimelineSim charges matmuls by OUTPUT FREE SIZE
only — contraction depth and output partitions are free):
  - scores per (head, kb): [key 128, q free] trimmed to the causal triangle.
  - AV runs TRANSPOSED: out [q 128, d 65] so each accumulation step costs 65
    rows instead of ~512; the ones column gives the softmax denominator.
  - The normalized token-major AV result is returned to feature-major layout
    with DMA-engine transposes (14ns/32x32 tile, off the compute engines).
  - exp for a head PAIR is fused into one Activation instruction (the two
    heads' score tiles sit in adjacent PSUM banks).
  - r = rsqrt(mean x^2) rides into Q via r-scaled RoPE tables, into scores
    via the per-key `scale` operand of exp, and into V via a per-partition
    tensor_scalar during the PSUM->SBUF copy. r_tok (token-major r) comes
    from 16 free PE transposes of the r row.
  - PE is kept continuously busy (the cost model halves PE speed after any
    idle gap until 3us of continuous execution): the K projection starts at
    xT chunk 2 so the DMA stream stays ahead of the PE stream.
"""

import os

import numpy as np
import ml_dtypes

BF16 = ml_dtypes.bfloat16

DIM = 1024
HEADS = 16
DIM_HEAD = 64
T = 2048  # tokens per batch
B = 2
HPC = 4  # heads per core
F = HPC * DIM_HEAD  # 256 per-core head width
KC = DIM // 128  # 8 contraction chunks
KORD = [2, 3, 4, 5, 6, 7, 0, 1]  # kc order: first matmul waits for chunk 2

_NC_CACHE = {}


def _build_nc():
    import concourse.bacc as bacc
    import concourse.mybir as mybir
    import concourse.tile as tile
    from contextlib import ExitStack

    f32 = mybir.dt.float32
    bf16 = mybir.dt.bfloat16
    nc = bacc.Bacc()

    xT = nc.declare_dram_parameter("xT", [DIM, T], bf16, isOutput=False)
    wq = nc.declare_dram_parameter("wq", [DIM, F], bf16, isOutput=False)
    wk = nc.declare_dram_parameter("wk", [DIM, F], bf16, isOutput=False)
    wv = nc.declare_dram_parameter("wv", [DIM, F], bf16, isOutput=False)
    wo = nc.declare_dram_parameter("wo", [F, DIM], bf16, isOutput=False)
    cosT = nc.declare_dram_parameter("cosT", [128, T], bf16, isOutput=False)
    sinT = nc.declare_dram_parameter("sinT", [128, T], bf16, isOutput=False)
    perm = nc.declare_dram_parameter("perm", [128, 128], bf16, isOutput=False)
    masks = nc.declare_dram_parameter("masks", [128, 128], bf16, isOutput=False)
    ident = nc.declare_dram_parameter("ident", [128, 128], bf16, isOutput=False)
    out = nc.declare_dram_parameter("out", [DIM, T], bf16, isOutput=True)
    tap = os.environ.get("KTAP", "")
    dbg = None
    if tap:
        _tap_shapes = {
            "rtok": ([128, 16], f32),
            "qk": ([128, 4, T], bf16),
            "v": ([128, 16, HPC, 65], bf16),
            "avtok": ([128, 16, F], bf16),
            "avall": ([128, 2, T], bf16),
        }
        shp, dt = _tap_shapes[tap]
        dbg = nc.declare_dram_parameter("dbg", shp, dt, isOutput=True)

    Exp = mybir.ActivationFunctionType.Exp
    Sqrt = mybir.ActivationFunctionType.Sqrt
    mult = mybir.AluOpType.mult
    add = mybir.AluOpType.add

    with ExitStack() as ctx:
        tc = ctx.enter_context(tile.TileContext(nc))
        consts = ctx.enter_context(tc.tile_pool(name="consts", bufs=1))
        persist = ctx.enter_context(tc.tile_pool(name="persist", bufs=1))
        work = ctx.enter_context(tc.tile_pool(name="work", bufs=4))
        vecs = ctx.enter_context(tc.tile_pool(name="vecs", bufs=1))

        # ---- constants / inputs ----
        wk_sb = consts.tile([128, KC, F], bf16, tag="wk")
        wq_sb = consts.tile([128, KC, F], bf16, tag="wq")
        wv_sb = consts.tile([128, KC, F], bf16, tag="wv")
        wo_sb = consts.tile([128, 2, DIM], bf16, tag="wo")
        cos_sb = consts.tile([128, T], bf16, tag="cos")
        sin_sb = consts.tile([128, T], bf16, tag="sin")
        perm_sb = consts.tile([128, 128], bf16, tag="perm")
        mask_sb = consts.tile([128, 128], bf16, tag="mask")
        id_sb = consts.tile([128, 128], bf16, tag="ident")
        ones_col = consts.tile([128, 1], bf16, tag="onesc")
        one_f32 = consts.tile([1, 1], f32, tag="onef")
        xT_sb = persist.tile([128, KC, T], bf16, tag="xT")
        xT_r = xT.rearrange("(kc p) t -> p kc t", p=128)
        # wk first (first PE consumer), then xT chunks in consumption order
        # with the other weights slotted behind the early chunks
        nc.sync.dma_start(wk_sb, wk.rearrange("(kc p) f -> p kc f", p=128))
        for kc in KORD[:4]:
            nc.sync.dma_start(xT_sb[:, kc], xT_r[:, kc])
        nc.sync.dma_start(perm_sb, perm[:, :])
        nc.sync.dma_start(cos_sb, cosT[:, :])
        nc.sync.dma_start(sin_sb, sinT[:, :])
        for kc in KORD[4:]:
            nc.sync.dma_start(xT_sb[:, kc], xT_r[:, kc])
        nc.sync.dma_start(wq_sb, wq.rearrange("(kc p) f -> p kc f", p=128))
        nc.sync.dma_start(wv_sb, wv.rearrange("(kc p) f -> p kc f", p=128))
        nc.sync.dma_start(mask_sb, masks[:, :])
        nc.sync.dma_start(id_sb, ident[:, :])
        nc.sync.dma_start(wo_sb, wo.rearrange("(fc p) d -> p fc d", p=128))
        nc.vector.memset(ones_col, 1.0)
        nc.vector.memset(one_f32, 1.0)

        # persistent activations
        qk_sb = persist.tile([128, 4, T], bf16, tag="qk")  # 0,1=q fc0/1; 2,3=k
        v_sb = persist.tile([128, 16, HPC, 65], bf16, tag="v")
        av_tok = persist.tile([128, 16, F], bf16, tag="avtok")
        av_all = persist.tile([128, 2, T], bf16, tag="av")
        r_sb = vecs.tile([1, T], f32, tag="r")
        r_tok = vecs.tile([128, 16], f32, tag="rtok")
        r_bc = persist.tile([128, T], f32, tag="rbc")
        cosr_sb = persist.tile([128, T], bf16, tag="cosr")
        sinr_sb = persist.tile([128, T], bf16, tag="sinr")
        nc.vector.memset(v_sb[:, :, :, 64:65], 1.0)

        ctxA = ExitStack()
        psKQ = ctxA.enter_context(tc.tile_pool(name="psKQ", bufs=8, space="PSUM"))
        sbA = ctxA.enter_context(tc.tile_pool(name="sbA", bufs=1))
        xsq_sb = sbA.tile([128, KC, T], bf16, tag="xsq")

        # x^2 per chunk (DVE, chases the xT DMAs)
        for kc in KORD:
            nc.vector.tensor_mul(xsq_sb[:, kc], xT_sb[:, kc], xT_sb[:, kc])

        def proj_rope(fidx, psum_tiles, is_q):
            """Finish a Q/K projection: PSUM->SBUF copy, rotate-half perm
            matmul, rope multiply-adds into qk_sb[fidx]. Q uses the r-scaled
            tables so r_q rides in for free."""
            cc = cosr_sb if is_q else cos_sb
            ssb = sinr_sb if is_q else sin_sb
            for tt in range(4):
                ts = slice(tt * 512, (tt + 1) * 512)
                raw = work.tile([128, 512], bf16, tag="raw")
                if is_q or tt % 2 == 0:
                    nc.scalar.copy(out=raw, in_=psum_tiles[tt])
                else:
                    nc.vector.tensor_copy(out=raw, in_=psum_tiles[tt])
                pp = psKQ.tile([128, 512], f32, tag="proj", name=f"pp_{fidx}_{tt}")
                nc.tensor.matmul(pp, lhsT=perm_sb, rhs=raw, start=True, stop=True)
                t1 = work.tile([128, 512], bf16, tag="t1")
                nc.vector.tensor_tensor(t1, pp, ssb[:, ts], mult)
                t2 = work.tile([128, 512], bf16, tag="t2")
                nc.vector.tensor_tensor(t2, raw, cc[:, ts], mult)
                if is_q:
                    nc.vector.tensor_tensor(qk_sb[:, fidx, ts], t2, t1, add)
                else:
                    nc.gpsimd.tensor_tensor(qk_sb[:, fidx, ts], t2, t1, add)

        # ---- K projection (both fc), chunk-paced off the xT DMA stream ----
        psK = {}
        for fc in range(2):
            for tt in range(4):
                psK[(fc, tt)] = psKQ.tile(
                    [128, 512], f32, tag="proj", name=f"k_{fc}_{tt}"
                )
        for kc in KORD:
            for fc in range(2):
                for tt in range(4):
                    nc.tensor.matmul(
                        psK[(fc, tt)],
                        lhsT=wk_sb[:, kc, fc * 128 : (fc + 1) * 128],
                        rhs=xT_sb[:, kc, tt * 512 : (tt + 1) * 512],
                        start=(kc == KORD[0]),
                        stop=(kc == KORD[-1]),
                    )
        for fc in range(2):
            proj_rope(2 + fc, [psK[(fc, tt)] for tt in range(4)], False)

        # ---- sum(x^2) ones-matmuls with the r-chain pipelined per slice ----
        ss_sb = sbA.tile([1, T], f32, tag="ss")
        sq_sb = sbA.tile([1, T], f32, tag="sq")
        for s in range(4):
            ts = slice(s * 512, (s + 1) * 512)
            ss_ps = psKQ.tile([1, 512], f32, tag="proj", name=f"ss_{s}")
            for kc in range(KC):
                nc.tensor.matmul(
                    ss_ps,
                    lhsT=ones_col,
                    rhs=xsq_sb[:, kc, s * 512 : (s + 1) * 512],
                    start=(kc == 0),
                    stop=(kc == KC - 1),
                )
            nc.scalar.copy(out=ss_sb[:, ts], in_=ss_ps)
            nc.scalar.activation(sq_sb[:, ts], ss_sb[:, ts], Sqrt, scale=1.0 / DIM)
            nc.vector.reciprocal(r_sb[:, ts], sq_sb[:, ts])
            nc.gpsimd.partition_broadcast(r_bc[:, ts], r_sb[:, ts])
            nc.gpsimd.tensor_tensor(cosr_sb[:, ts], cos_sb[:, ts], r_bc[:, ts], mult)
            nc.gpsimd.tensor_tensor(sinr_sb[:, ts], sin_sb[:, ts], r_bc[:, ts], mult)
        # Q fc0 projection
        psQ0 = [
            psKQ.tile([128, 512], f32, tag="proj", name=f"q0_{tt}")
            for tt in range(4)
        ]
        for kc in range(KC):
            for tt in range(4):
                nc.tensor.matmul(
                    psQ0[tt],
                    lhsT=wq_sb[:, kc, 0:128],
                    rhs=xT_sb[:, kc, tt * 512 : (tt + 1) * 512],
                    start=(kc == 0),
                    stop=(kc == KC - 1),
                )
        # r_tok via PE transposes of the r row
        rtok_ps = psKQ.tile([128, 16], f32, tag="proj", name="rtokps")
        for i in range(16):
            nc.tensor.transpose(
                rtok_ps[:, i : i + 1], r_sb[0:1, i * 128 : (i + 1) * 128],
                one_f32,
            )
        nc.vector.tensor_copy(out=r_tok, in_=rtok_ps)

        proj_rope(0, psQ0, True)

        # ---- Q fc1 ----
        psQ1 = [
            psKQ.tile([128, 512], f32, tag="proj", name=f"q1_{tt}")
            for tt in range(4)
        ]
        for kc in range(KC):
            for tt in range(4):
                nc.tensor.matmul(
                    psQ1[tt],
                    lhsT=wq_sb[:, kc, 128:256],
                    rhs=xT_sb[:, kc, tt * 512 : (tt + 1) * 512],
                    start=(kc == 0),
                    stop=(kc == KC - 1),
                )
        proj_rope(1, psQ1, True)

        # ---- V projection (token-major) + r_tok scaling ----
        ctxA.close()

        # ---- attention: scores [k,q] -> paired exp -> transposed AV ----
        # PSUM: sc ring (2x2 banks, also V-proj) + av4/po ring (3) + avT (1)
        with (
            tc.tile_pool(name="psSC", bufs=2, space="PSUM") as psSC,
            tc.tile_pool(name="psAV", bufs=3, space="PSUM") as psAV,
            tc.tile_pool(name="psT", bufs=1, space="PSUM") as psT,
            tc.tile_pool(name="expp", bufs=4) as expp,
            tc.tile_pool(name="recp", bufs=4) as recp,
        ):
            def v_proj(tt):
                psv = psSC.tile([128, 256], f32, tag="sc", name=f"v_{tt}")
                for kc in range(KC):
                    nc.tensor.matmul(
                        psv,
                        lhsT=xT_sb[:, kc, tt * 128 : (tt + 1) * 128],
                        rhs=wv_sb[:, kc, :],
                        start=(kc == 0),
                        stop=(kc == KC - 1),
                    )
                nc.vector.tensor_scalar(
                    out=v_sb[:, tt, :, 0:64],
                    in0=psv.rearrange("p (h d) -> p h d", h=HPC),
                    scalar1=r_tok[:, tt : tt + 1],
                    scalar2=None,
                    op0=mult,
                )

            for qt in range(4):
                q0 = qt * 512
                for pi in range(2):
                    # full-bank tiles: matmul start=True zeroes the whole 2KB
                    # bank, so only the FIRST write into each bank uses it
                    av4 = [
                        psAV.tile([128, 4, 128], f32, tag="av4",
                                  name=f"av_{qt}_{pi}_{x}")
                        for x in range(2)
                    ]
                    def emit_av(kb, ex):
                        for qbl in range(4):
                            qb = 4 * qt + qbl
                            if kb > qb:
                                continue
                            for x in range(2):
                                nc.tensor.matmul(
                                    av4[x][:, qbl, 0:65],
                                    lhsT=ex[:, x * 512 + qbl * 128 : x * 512 + (qbl + 1) * 128],
                                    rhs=v_sb[:, kb, 2 * pi + x, :],
                                    start=(kb == 0 and qbl == 0),
                                    stop=(kb == qb),
                                    skip_group_check=True,
                                )

                    nkb = 4 * qt + 4
                    pend = None
                    for kb in range(nkb + 1):
                        cur = None
                        if kb < nkb:
                            # stagger this quarter's V projections into pair
                            # 0's kb loop so PE fills gaps while Act runs exp
                            if pi == 0 and qt == 0:
                                v_proj(kb)
                            c0 = max(0, kb * 128 - q0)
                            sc = psSC.tile(
                                [128, 1024], f32, tag="sc",
                                name=f"sc_{qt}_{pi}_{kb}"
                            )
                            for x in range(2):
                                rX = slice(x * 64, x * 64 + 64)
                                nc.tensor.matmul(
                                    sc[:, x * 512 + c0 : x * 512 + 512],
                                    lhsT=qk_sb[rX, 2 + pi, kb * 128 : (kb + 1) * 128],
                                    rhs=qk_sb[rX, pi, q0 + c0 : q0 + 512],
                                    start=True,
                                    stop=True,
                                )
                            if pi == 0 and qt > 0 and kb < 4:
                                v_proj(4 * qt + kb)
                            ex = expp.tile([128, 1024], bf16, tag="exp")
                            if c0 == 0:
                                nc.scalar.activation(
                                    ex, sc, Exp, scale=r_tok[:, kb : kb + 1]
                                )
                            else:
                                for x in range(2):
                                    nc.scalar.activation(
                                        ex[:, x * 512 + c0 : x * 512 + 512],
                                        sc[:, x * 512 + c0 : x * 512 + 512],
                                        Exp,
                                        scale=r_tok[:, kb : kb + 1],
                                    )
                            if kb >= 4 * qt:  # diagonal block: causal mask
                                for x in range(2):
                                    nc.gpsimd.tensor_tensor(
                                        ex[:, x * 512 + c0 : x * 512 + c0 + 128],
                                        ex[:, x * 512 + c0 : x * 512 + c0 + 128],
                                        mask_sb,
                                        mult,
                                    )
                            cur = (kb, ex)
                        if pend is not None:
                            emit_av(*pend)
                        pend = cur
                    # normalize (rows 0..63 / row 64) into token-major av_tok
                    for x in range(2):
                        h = 2 * pi + x
                        rec4 = recp.tile([128, 4], f32, tag="rec")
                        nc.vector.reciprocal(rec4, av4[x][:, :, 64:65])
                        for qbl in range(4):
                            nc.vector.tensor_scalar(
                                out=av_tok[:, 4 * qt + qbl, h * 64 : (h + 1) * 64],
                                in0=av4[x][:, qbl, 0:64],
                                scalar1=rec4[:, qbl : qbl + 1],
                                scalar2=None,
                                op0=mult,
                            )
                # back to feature-major via PE transposes (53ns each)
                avT = psT.tile([128, 8, 128], bf16, tag="avt", name=f"avt_{qt}")
                for j, tt in enumerate(range(4 * qt, 4 * qt + 4)):
                    for fc in range(2):
                        nc.tensor.transpose(
                            avT[:, fc * 4 + j, :],
                            av_tok[:, tt, fc * 128 : (fc + 1) * 128],
                            id_sb,
                        )
                for fc in range(2):
                    nc.vector.tensor_copy(
                        out=av_all[:, fc, q0 : q0 + 512],
                        in_=avT[:, fc * 4 : fc * 4 + 4, :],
                    )
                # out-projection for this token quarter
                for do in range(8):
                    po = psAV.tile([128, 512], f32, tag="av4", name=f"o_{qt}_{do}")
                    for fc in range(2):
                        nc.tensor.matmul(
                            po,
                            lhsT=wo_sb[:, fc, do * 128 : (do + 1) * 128],
                            rhs=av_all[:, fc, q0 : q0 + 512],
                            start=(fc == 0),
                            stop=(fc == 1),
                        )
                    ob = work.tile([128, 512], bf16, tag="ob")
                    nc.vector.tensor_copy(out=ob, in_=po)
                    nc.sync.dma_start(
                        out.rearrange("(do p) t -> p do t", p=128)[:, do, q0 : q0 + 512],
                        ob,
                    )
            if tap == "rtok":
                nc.sync.dma_start(dbg[:, :], r_tok)
            elif tap == "qk":
                nc.sync.dma_start(dbg[:, :, :], qk_sb)
            elif tap == "v":
                nc.sync.dma_start(dbg[:, :, :, :], v_sb)
            elif tap == "avtok":
                nc.sync.dma_start(dbg[:, :, :], av_tok)
            elif tap == "avall":
                nc.sync.dma_start(dbg[:, :, :], av_all)
    nc.compile()
    return nc


def _host_inputs(x, norm_w, w_qkv, w_o, sin, cos):
    """Build the 8 per-core input maps (all bf16)."""
    n = T
    w_eff = np.asarray(w_qkv, np.float64) * np.asarray(norm_w, np.float64)[:, None]
    sin_n = np.asarray(sin, np.float32)[:n]  # [T, 64]
    cos_n = np.asarray(cos, np.float32)[:n]
    sign = np.concatenate([-np.ones(32, np.float32), np.ones(32, np.float32)])
    cos_tile = np.tile(cos_n.T, (2, 1))  # [128, T]
    sin_tile = np.tile((sin_n * sign[None, :]).T, (2, 1))  # [128, T]
    perm = np.zeros((128, 128), np.float32)
    for m in range(128):
        d = m % 64
        k = m + 32 if d < 32 else m - 32
        perm[k, m] = 1.0
    ident_np = np.eye(128, dtype=np.float32)
    ql = np.arange(128)[None, :]
    key = np.arange(128)[:, None]
    masks = (ql >= key).astype(np.float32)

    in_maps = []
    for c in range(8):
        b, g = c // 4, c % 4
        fs = slice(g * F, (g + 1) * F)
        in_maps.append(
            {
                "xT": np.ascontiguousarray(np.asarray(x, np.float32)[b].T).astype(BF16),
                "wq": (w_eff[:, 0:DIM][:, fs] * (DIM_HEAD ** -0.5)).astype(BF16),
                "wk": w_eff[:, DIM : 2 * DIM][:, fs].astype(BF16),
                "wv": w_eff[:, 2 * DIM : 3 * DIM][:, fs].astype(BF16),
                "wo": np.asarray(w_o, np.float32)[fs, :].astype(BF16),
                "cosT": cos_tile.astype(BF16),
                "sinT": sin_tile.astype(BF16),
                "perm": perm.astype(BF16),
                "masks": masks.astype(BF16),
                "ident": ident_np.astype(BF16),
            }
        )
    return in_maps


def kernel(x, norm_w, w_qkv, w_o, b_o, sin, cos):
    from concourse.bass_utils import run_bass_kernel_spmd

    if "nc" not in _NC_CACHE:
        _NC_CACHE["nc"] = _build_nc()
    nc = _NC_CACHE["nc"]
    in_maps = _host_inputs(x, norm_w, w_qkv, w_o, sin, cos)
    trace = bool(int(os.environ.get("KERNEL_TRACE", "0")))
    res = run_bass_kernel_spmd(nc, in_maps, core_ids=list(range(8)), trace=trace)
    if trace and res.exec_time_ns is not None:
        print(f"HW exec time: {res.exec_time_ns} ns")
    outs = [r["out"].astype(np.float32) for r in res.results]  # [1024, T] fm
    b_o = np.asarray(b_o, np.float32)
    full = np.empty((B, T, DIM), np.float32)
    for b in range(B):
        acc = outs[b * 4] + outs[b * 4 + 1] + outs[b * 4 + 2] + outs[b * 4 + 3]
        full[b] = acc.T + b_o[None, :]
    return full


# revision 40
# speedup vs baseline: 1.2314x; 1.0326x over previous
"""Trainium2 8-core kernel for RMSNorm -> QKV -> RoPE -> causal SDPA -> out-proj.

Sharding: core c = b*4 + g handles batch b (of 2) and heads 4g..4g+3 (of 16).
Each core computes a partial out-projection [dim, tokens]; the host sums the
4 head-group partials per batch (the tensor-parallel "unshard") and adds b_o.

Cost-model-driven layout (TimelineSim charges matmuls by OUTPUT FREE SIZE
only — contraction depth and output partitions are free):
  - scores per (head, kb): [key 128, q free] trimmed to the causal triangle.
  - AV runs TRANSPOSED: out [q 128, d 65] so each accumulation step costs 65
    rows instead of ~512; the ones column gives the softmax denominator.
  - The normalized token-major AV result is returned to feature-major layout
    with DMA-engine transposes (14ns/32x32 tile, off the compute engines).
  - exp for a head PAIR is fused into one Activation instruction (the two
    heads' score tiles sit in adjacent PSUM banks).
  - r = rsqrt(mean x^2) rides into Q via r-scaled RoPE tables, into scores
    via the per-key `scale` operand of exp, and into V via a per-partition
    tensor_scalar during the PSUM->SBUF copy. r_tok (token-major r) comes
    from 16 free PE transposes of the r row.
  - PE is kept continuously busy (the cost model halves PE speed after any
    idle gap until 3us of continuous execution): the K projection starts at
    xT chunk 2 so the DMA stream stays ahead of the PE stream.
"""

import os

import numpy as np
import ml_dtypes

BF16 = ml_dtypes.bfloat16

DIM = 1024
HEADS = 16
DIM_HEAD = 64
T = 2048  # tokens per batch
B = 2
HPC = 4  # heads per core
F = HPC * DIM_HEAD  # 256 per-core head width
KC = DIM // 128  # 8 contraction chunks
KORD = [2, 3, 4, 5, 6, 7, 0, 1]  # kc order: first matmul waits for chunk 2

_NC_CACHE = {}


def _build_nc():
    import concourse.bacc as bacc
    import concourse.mybir as mybir
    import concourse.tile as tile
    from contextlib import ExitStack

    f32 = mybir.dt.float32
    bf16 = mybir.dt.bfloat16
    nc = bacc.Bacc()

    xT = nc.declare_dram_parameter("xT", [DIM, T], bf16, isOutput=False)
    wq = nc.declare_dram_parameter("wq", [DIM, F], bf16, isOutput=False)
    wk = nc.declare_dram_parameter("wk", [DIM, F], bf16, isOutput=False)
    wv = nc.declare_dram_parameter("wv", [DIM, F], bf16, isOutput=False)
    wo = nc.declare_dram_parameter("wo", [F, DIM], bf16, isOutput=False)
    cosT = nc.declare_dram_parameter("cosT", [128, T], bf16, isOutput=False)
    sinT = nc.declare_dram_parameter("sinT", [128, T], bf16, isOutput=False)
    perm = nc.declare_dram_parameter("perm", [128, 128], bf16, isOutput=False)
    masks = nc.declare_dram_parameter("masks", [128, 128], bf16, isOutput=False)
    ident = nc.declare_dram_parameter("ident", [128, 128], bf16, isOutput=False)
    out = nc.declare_dram_parameter("out", [DIM, T], bf16, isOutput=True)
    tap = os.environ.get("KTAP", "")
    dbg = None
    if tap:
        _tap_shapes = {
            "rtok": ([128, 16], f32),
            "qk": ([128, 4, T], bf16),
            "v": ([128, 16, HPC, 65], bf16),
            "avtok": ([128, 16, F], bf16),
            "avall": ([128, 2, T], bf16),
        }
        shp, dt = _tap_shapes[tap]
        dbg = nc.declare_dram_parameter("dbg", shp, dt, isOutput=True)

    Exp = mybir.ActivationFunctionType.Exp
    Sqrt = mybir.ActivationFunctionType.Sqrt
    mult = mybir.AluOpType.mult
    add = mybir.AluOpType.add

    with ExitStack() as ctx:
        tc = ctx.enter_context(tile.TileContext(nc))
        consts = ctx.enter_context(tc.tile_pool(name="consts", bufs=1))
        persist = ctx.enter_context(tc.tile_pool(name="persist", bufs=1))
        work = ctx.enter_context(tc.tile_pool(name="work", bufs=4))
        vecs = ctx.enter_context(tc.tile_pool(name="vecs", bufs=1))

        # ---- constants / inputs ----
        wk_sb = consts.tile([128, KC, F], bf16, tag="wk")
        wq_sb = consts.tile([128, KC, F], bf16, tag="wq")
        wv_sb = consts.tile([128, KC, F], bf16, tag="wv")
        wo_sb = consts.tile([128, 2, DIM], bf16, tag="wo")
        cos_sb = consts.tile([128, T], bf16, tag="cos")
        sin_sb = consts.tile([128, T], bf16, tag="sin")
        perm_sb = consts.tile([128, 128], bf16, tag="perm")
        mask_sb = consts.tile([128, 128], bf16, tag="mask")
        id_sb = consts.tile([128, 128], bf16, tag="ident")
        ones_col = consts.tile([128, 1], bf16, tag="onesc")
        one_f32 = consts.tile([1, 1], f32, tag="onef")
        xT_sb = persist.tile([128, KC, T], bf16, tag="xT")
        xT_r = xT.rearrange("(kc p) t -> p kc t", p=128)
        # wk first (first PE consumer), then xT chunks in consumption order
        # with the other weights slotted behind the early chunks
        nc.sync.dma_start(wk_sb, wk.rearrange("(kc p) f -> p kc f", p=128))
        for kc in KORD[:4]:
            nc.sync.dma_start(xT_sb[:, kc], xT_r[:, kc])
        nc.sync.dma_start(perm_sb, perm[:, :])
        nc.sync.dma_start(cos_sb, cosT[:, :])
        nc.sync.dma_start(sin_sb, sinT[:, :])
        for kc in KORD[4:]:
            nc.sync.dma_start(xT_sb[:, kc], xT_r[:, kc])
        nc.sync.dma_start(wq_sb, wq.rearrange("(kc p) f -> p kc f", p=128))
        nc.sync.dma_start(wv_sb, wv.rearrange("(kc p) f -> p kc f", p=128))
        nc.sync.dma_start(mask_sb, masks[:, :])
        nc.sync.dma_start(id_sb, ident[:, :])
        nc.sync.dma_start(wo_sb, wo.rearrange("(fc p) d -> p fc d", p=128))
        nc.vector.memset(ones_col, 1.0)
        nc.vector.memset(one_f32, 1.0)

        # persistent activations
        qk_sb = persist.tile([128, 4, T], bf16, tag="qk")  # 0,1=q fc0/1; 2,3=k
        v_sb = persist.tile([128, 16, HPC, 65], bf16, tag="v")
        av_tok = persist.tile([128, 16, F], bf16, tag="avtok")
        av_all = persist.tile([128, 2, T], bf16, tag="av")
        r_sb = vecs.tile([1, T], f32, tag="r")
        r_tok = vecs.tile([128, 16], f32, tag="rtok")
        r_bc = persist.tile([128, T], f32, tag="rbc")
        cosr_sb = persist.tile([128, T], bf16, tag="cosr")
        sinr_sb = persist.tile([128, T], bf16, tag="sinr")
        nc.vector.memset(v_sb[:, :, :, 64:65], 1.0)

        ctxA = ExitStack()
        psKQ = ctxA.enter_context(tc.tile_pool(name="psKQ", bufs=8, space="PSUM"))
        sbA = ctxA.enter_context(tc.tile_pool(name="sbA", bufs=1))
        xsq_sb = sbA.tile([128, KC, T], bf16, tag="xsq")

        # x^2 per chunk (DVE, chases the xT DMAs)
        for kc in KORD:
            nc.vector.tensor_mul(xsq_sb[:, kc], xT_sb[:, kc], xT_sb[:, kc])

        def proj_rope(fidx, psum_tiles, is_q):
            """Finish a Q/K projection: PSUM->SBUF copy, rotate-half perm
            matmul, rope multiply-adds into qk_sb[fidx]. Q uses the r-scaled
            tables so r_q rides in for free."""
            cc = cosr_sb if is_q else cos_sb
            ssb = sinr_sb if is_q else sin_sb
            for tt in range(4):
                ts = slice(tt * 512, (tt + 1) * 512)
                raw = work.tile([128, 512], bf16, tag="raw")
                if is_q or tt % 2 == 0:
                    nc.scalar.copy(out=raw, in_=psum_tiles[tt])
                else:
                    nc.vector.tensor_copy(out=raw, in_=psum_tiles[tt])
                pp = psKQ.tile([128, 512], f32, tag="proj", name=f"pp_{fidx}_{tt}")
                nc.tensor.matmul(pp, lhsT=perm_sb, rhs=raw, start=True, stop=True)
                t1 = work.tile([128, 512], bf16, tag="t1")
                nc.vector.tensor_tensor(t1, pp, ssb[:, ts], mult)
                t2 = work.tile([128, 512], bf16, tag="t2")
                nc.vector.tensor_tensor(t2, raw, cc[:, ts], mult)
                if is_q:
                    nc.vector.tensor_tensor(qk_sb[:, fidx, ts], t2, t1, add)
                else:
                    nc.gpsimd.tensor_tensor(qk_sb[:, fidx, ts], t2, t1, add)

        # ---- K projection (both fc), chunk-paced off the xT DMA stream ----
        psK = {}
        for fc in range(2):
            for tt in range(4):
                psK[(fc, tt)] = psKQ.tile(
                    [128, 512], f32, tag="proj", name=f"k_{fc}_{tt}"
                )
        for kc in KORD:
            for fc in range(2):
                for tt in range(4):
                    nc.tensor.matmul(
                        psK[(fc, tt)],
                        lhsT=wk_sb[:, kc, fc * 128 : (fc + 1) * 128],
                        rhs=xT_sb[:, kc, tt * 512 : (tt + 1) * 512],
                        start=(kc == KORD[0]),
                        stop=(kc == KORD[-1]),
                    )
        for fc in range(2):
            proj_rope(2 + fc, [psK[(fc, tt)] for tt in range(4)], False)

        # ---- sum(x^2) ones-matmuls with the r-chain pipelined per slice ----
        ss_sb = sbA.tile([1, T], f32, tag="ss")
        sq_sb = sbA.tile([1, T], f32, tag="sq")
        for s in range(4):
            ts = slice(s * 512, (s + 1) * 512)
            ss_ps = psKQ.tile([1, 512], f32, tag="proj", name=f"ss_{s}")
            for kc in range(KC):
                nc.tensor.matmul(
                    ss_ps,
                    lhsT=ones_col,
                    rhs=xsq_sb[:, kc, s * 512 : (s + 1) * 512],
                    start=(kc == 0),
                    stop=(kc == KC - 1),
                )
            nc.scalar.copy(out=ss_sb[:, ts], in_=ss_ps)
            nc.scalar.activation(sq_sb[:, ts], ss_sb[:, ts], Sqrt, scale=1.0 / DIM)
            nc.vector.reciprocal(r_sb[:, ts], sq_sb[:, ts])
            nc.gpsimd.partition_broadcast(r_bc[:, ts], r_sb[:, ts])
            nc.gpsimd.tensor_tensor(cosr_sb[:, ts], cos_sb[:, ts], r_bc[:, ts], mult)
            nc.gpsimd.tensor_tensor(sinr_sb[:, ts], sin_sb[:, ts], r_bc[:, ts], mult)
        # Q fc0 projection
        psQ0 = [
            psKQ.tile([128, 512], f32, tag="proj", name=f"q0_{tt}")
            for tt in range(4)
        ]
        for kc in range(KC):
            for tt in range(4):
                nc.tensor.matmul(
                    psQ0[tt],
                    lhsT=wq_sb[:, kc, 0:128],
                    rhs=xT_sb[:, kc, tt * 512 : (tt + 1) * 512],
                    start=(kc == 0),
                    stop=(kc == KC - 1),
                )
        # r_tok via PE transposes of the r row
        rtok_ps = psKQ.tile([128, 16], f32, tag="proj", name="rtokps")
        for i in range(16):
            nc.tensor.transpose(
                rtok_ps[:, i : i + 1], r_sb[0:1, i * 128 : (i + 1) * 128],
                one_f32,
            )
        nc.vector.tensor_copy(out=r_tok, in_=rtok_ps)

        proj_rope(0, psQ0, True)

        # ---- Q fc1 ----
        psQ1 = [
            psKQ.tile([128, 512], f32, tag="proj", name=f"q1_{tt}")
            for tt in range(4)
        ]
        for kc in range(KC):
            for tt in range(4):
                nc.tensor.matmul(
                    psQ1[tt],
                    lhsT=wq_sb[:, kc, 128:256],
                    rhs=xT_sb[:, kc, tt * 512 : (tt + 1) * 512],
                    start=(kc == 0),
                    stop=(kc == KC - 1),
                )
        proj_rope(1, psQ1, True)

        # ---- V projection (token-major) + r_tok scaling ----
        ctxA.close()

        # ---- attention: scores [k,q] -> paired exp -> transposed AV ----
        # PSUM: sc ring (2x2 banks, also V-proj) + av4/po ring (3) + avT (1)
        with (
            tc.tile_pool(name="psSC", bufs=3, space="PSUM") as psSC,
            tc.tile_pool(name="psAV", bufs=2, space="PSUM") as psAV,
            tc.tile_pool(name="expp", bufs=6) as expp,
            tc.tile_pool(name="recp", bufs=4) as recp,
        ):
            def v_proj(tt):
                psv = psSC.tile([128, 256], f32, tag="sc", name=f"v_{tt}")
                for kc in range(KC):
                    nc.tensor.matmul(
                        psv,
                        lhsT=xT_sb[:, kc, tt * 128 : (tt + 1) * 128],
                        rhs=wv_sb[:, kc, :],
                        start=(kc == 0),
                        stop=(kc == KC - 1),
                    )
                nc.vector.tensor_scalar(
                    out=v_sb[:, tt, :, 0:64],
                    in0=psv.rearrange("p (h d) -> p h d", h=HPC),
                    scalar1=r_tok[:, tt : tt + 1],
                    scalar2=None,
                    op0=mult,
                )

            for qt in range(4):
                q0 = qt * 512
                for pi in range(2):
                    # full-bank tiles: matmul start=True zeroes the whole 2KB
                    # bank, so only the FIRST write into each bank uses it
                    av4 = [
                        psAV.tile([128, 4, 128], f32, tag="av4",
                                  name=f"av_{qt}_{pi}_{x}")
                        for x in range(2)
                    ]
                    def emit_av(kb, ex):
                        for qbl in range(4):
                            qb = 4 * qt + qbl
                            if kb > qb:
                                continue
                            for x in range(2):
                                nc.tensor.matmul(
                                    av4[x][:, qbl, 0:65],
                                    lhsT=ex[:, x * 512 + qbl * 128 : x * 512 + (qbl + 1) * 128],
                                    rhs=v_sb[:, kb, 2 * pi + x, :],
                                    start=(kb == 0 and qbl == 0),
                                    stop=(kb == qb),
                                    skip_group_check=True,
                                )

                    nkb = 4 * qt + 4
                    pend = None
                    for kb in range(nkb + 1):
                        cur = None
                        if kb < nkb:
                            # stagger this quarter's V projections into pair
                            # 0's kb loop so PE fills gaps while Act runs exp
                            if pi == 0 and qt == 0:
                                v_proj(kb)
                            c0 = max(0, kb * 128 - q0)
                            sc = psSC.tile(
                                [128, 1024], f32, tag="sc",
                                name=f"sc_{qt}_{pi}_{kb}"
                            )
                            for x in range(2):
                                rX = slice(x * 64, x * 64 + 64)
                                nc.tensor.matmul(
                                    sc[:, x * 512 + c0 : x * 512 + 512],
                                    lhsT=qk_sb[rX, 2 + pi, kb * 128 : (kb + 1) * 128],
                                    rhs=qk_sb[rX, pi, q0 + c0 : q0 + 512],
                                    start=True,
                                    stop=True,
                                )
                            if pi == 0 and qt > 0 and kb < 4:
                                v_proj(4 * qt + kb)
                            ex = expp.tile([128, 1024], bf16, tag="exp")
                            if c0 == 0:
                                nc.scalar.activation(
                                    ex, sc, Exp, scale=r_tok[:, kb : kb + 1]
                                )
                            else:
                                for x in range(2):
                                    nc.scalar.activation(
                                        ex[:, x * 512 + c0 : x * 512 + 512],
                                        sc[:, x * 512 + c0 : x * 512 + 512],
                                        Exp,
                                        scale=r_tok[:, kb : kb + 1],
                                    )
                            if kb >= 4 * qt:  # diagonal block: causal mask
                                for x in range(2):
                                    nc.gpsimd.tensor_tensor(
                                        ex[:, x * 512 + c0 : x * 512 + c0 + 128],
                                        ex[:, x * 512 + c0 : x * 512 + c0 + 128],
                                        mask_sb,
                                        mult,
                                    )
                            cur = (kb, ex)
                        if pend is not None:
                            emit_av(*pend)
                        pend = cur
                    # normalize (rows 0..63 / row 64) into token-major av_tok
                    for x in range(2):
                        h = 2 * pi + x
                        rec4 = recp.tile([128, 4], f32, tag="rec")
                        nc.vector.reciprocal(rec4, av4[x][:, :, 64:65])
                        for qbl in range(4):
                            nc.vector.tensor_scalar(
                                out=av_tok[:, 4 * qt + qbl, h * 64 : (h + 1) * 64],
                                in0=av4[x][:, qbl, 0:64],
                                scalar1=rec4[:, qbl : qbl + 1],
                                scalar2=None,
                                op0=mult,
                            )
                # back to feature-major via PE transposes (53ns each)
                avT = psSC.tile([128, 8, 128], bf16, tag="sc", name=f"avt_{qt}")
                for j, tt in enumerate(range(4 * qt, 4 * qt + 4)):
                    for fc in range(2):
                        nc.tensor.transpose(
                            avT[:, fc * 4 + j, :],
                            av_tok[:, tt, fc * 128 : (fc + 1) * 128],
                            id_sb,
                        )
                for fc in range(2):
                    nc.vector.tensor_copy(
                        out=av_all[:, fc, q0 : q0 + 512],
                        in_=avT[:, fc * 4 : fc * 4 + 4, :],
                    )
                # out-projection for this token quarter
                for do in range(8):
                    po = psAV.tile([128, 512], f32, tag="av4", name=f"o_{qt}_{do}")
                    for fc in range(2):
                        nc.tensor.matmul(
                            po,
                            lhsT=wo_sb[:, fc, do * 128 : (do + 1) * 128],
                            rhs=av_all[:, fc, q0 : q0 + 512],
                            start=(fc == 0),
                            stop=(fc == 1),
                        )
                    ob = work.tile([128, 512], bf16, tag="ob")
                    nc.vector.tensor_copy(out=ob, in_=po)
                    nc.sync.dma_start(
                        out.rearrange("(do p) t -> p do t", p=128)[:, do, q0 : q0 + 512],
                        ob,
                    )
            if tap == "rtok":
                nc.sync.dma_start(dbg[:, :], r_tok)
            elif tap == "qk":
                nc.sync.dma_start(dbg[:, :, :], qk_sb)
            elif tap == "v":
                nc.sync.dma_start(dbg[:, :, :, :], v_sb)
            elif tap == "avtok":
                nc.sync.dma_start(dbg[:, :, :], av_tok)
            elif tap == "avall":
                nc.sync.dma_start(dbg[:, :, :], av_all)
    nc.compile()
    return nc


def _host_inputs(x, norm_w, w_qkv, w_o, sin, cos):
    """Build the 8 per-core input maps (all bf16)."""
    n = T
    w_eff = np.asarray(w_qkv, np.float64) * np.asarray(norm_w, np.float64)[:, None]
    sin_n = np.asarray(sin, np.float32)[:n]  # [T, 64]
    cos_n = np.asarray(cos, np.float32)[:n]
    sign = np.concatenate([-np.ones(32, np.float32), np.ones(32, np.float32)])
    cos_tile = np.tile(cos_n.T, (2, 1))  # [128, T]
    sin_tile = np.tile((sin_n * sign[None, :]).T, (2, 1))  # [128, T]
    perm = np.zeros((128, 128), np.float32)
    for m in range(128):
        d = m % 64
        k = m + 32 if d < 32 else m - 32
        perm[k, m] = 1.0
    ident_np = np.eye(128, dtype=np.float32)
    ql = np.arange(128)[None, :]
    key = np.arange(128)[:, None]
    masks = (ql >= key).astype(np.float32)

    in_maps = []
    for c in range(8):
        b, g = c // 4, c % 4
        fs = slice(g * F, (g + 1) * F)
        in_maps.append(
            {
                "xT": np.ascontiguousarray(np.asarray(x, np.float32)[b].T).astype(BF16),
                "wq": (w_eff[:, 0:DIM][:, fs] * (DIM_HEAD ** -0.5)).astype(BF16),
                "wk": w_eff[:, DIM : 2 * DIM][:, fs].astype(BF16),
                "wv": w_eff[:, 2 * DIM : 3 * DIM][:, fs].astype(BF16),
                "wo": np.asarray(w_o, np.float32)[fs, :].astype(BF16),
                "cosT": cos_tile.astype(BF16),
                "sinT": sin_tile.astype(BF16),
                "perm": perm.astype(BF16),
                "masks": masks.astype(BF16),
                "ident": ident_np.astype(BF16),
            }
        )
    return in_maps


def kernel(x, norm_w, w_qkv, w_o, b_o, sin, cos):
    from concourse.bass_utils import run_bass_kernel_spmd

    if "nc" not in _NC_CACHE:
        _NC_CACHE["nc"] = _build_nc()
    nc = _NC_CACHE["nc"]
    in_maps = _host_inputs(x, norm_w, w_qkv, w_o, sin, cos)
    trace = bool(int(os.environ.get("KERNEL_TRACE", "0")))
    res = run_bass_kernel_spmd(nc, in_maps, core_ids=list(range(8)), trace=trace)
    if trace and res.exec_time_ns is not None:
        print(f"HW exec time: {res.exec_time_ns} ns")
    outs = [r["out"].astype(np.float32) for r in res.results]  # [1024, T] fm
    b_o = np.asarray(b_o, np.float32)
    full = np.empty((B, T, DIM), np.float32)
    for b in range(B):
        acc = outs[b * 4] + outs[b * 4 + 1] + outs[b * 4 + 2] + outs[b * 4 + 3]
        full[b] = acc.T + b_o[None, :]
    return full


# revision 43
# speedup vs baseline: 1.2549x; 1.0191x over previous
"""Trainium2 8-core kernel for RMSNorm -> QKV -> RoPE -> causal SDPA -> out-proj.

Sharding: core c = b*4 + g handles batch b (of 2) and heads 4g..4g+3 (of 16).
Each core computes a partial out-projection [dim, tokens]; the host sums the
4 head-group partials per batch (the tensor-parallel "unshard") and adds b_o.

Cost-model-driven layout (TimelineSim charges matmuls by OUTPUT FREE SIZE
only — contraction depth and output partitions are free):
  - scores per (head, kb): [key 128, q free] trimmed to the causal triangle.
  - AV runs TRANSPOSED: out [q 128, d 65] so each accumulation step costs 65
    rows instead of ~512; the ones column gives the softmax denominator.
  - The normalized token-major AV result is returned to feature-major layout
    with DMA-engine transposes (14ns/32x32 tile, off the compute engines).
  - exp for a head PAIR is fused into one Activation instruction (the two
    heads' score tiles sit in adjacent PSUM banks).
  - r = rsqrt(mean x^2) rides into Q via r-scaled RoPE tables, into scores
    via the per-key `scale` operand of exp, and into V via a per-partition
    tensor_scalar during the PSUM->SBUF copy. r_tok (token-major r) comes
    from 16 free PE transposes of the r row.
  - PE is kept continuously busy (the cost model halves PE speed after any
    idle gap until 3us of continuous execution): the K projection starts at
    xT chunk 2 so the DMA stream stays ahead of the PE stream.
"""

import os

import numpy as np
import ml_dtypes

BF16 = ml_dtypes.bfloat16

DIM = 1024
HEADS = 16
DIM_HEAD = 64
T = 2048  # tokens per batch
B = 2
HPC = 4  # heads per core
F = HPC * DIM_HEAD  # 256 per-core head width
KC = DIM // 128  # 8 contraction chunks
KORD = [2, 3, 4, 5, 6, 7, 0, 1]  # kc order: first matmul waits for chunk 2

_NC_CACHE = {}


def _build_nc():
    import concourse.bacc as bacc
    import concourse.mybir as mybir
    import concourse.tile as tile
    from contextlib import ExitStack

    f32 = mybir.dt.float32
    bf16 = mybir.dt.bfloat16
    nc = bacc.Bacc()

    xT = nc.declare_dram_parameter("xT", [DIM, T], bf16, isOutput=False)
    wq = nc.declare_dram_parameter("wq", [DIM, F], bf16, isOutput=False)
    wk = nc.declare_dram_parameter("wk", [DIM, F], bf16, isOutput=False)
    wv = nc.declare_dram_parameter("wv", [DIM, F], bf16, isOutput=False)
    wo = nc.declare_dram_parameter("wo", [F, DIM], bf16, isOutput=False)
    cosT = nc.declare_dram_parameter("cosT", [128, T], bf16, isOutput=False)
    sinT = nc.declare_dram_parameter("sinT", [128, T], bf16, isOutput=False)
    perm = nc.declare_dram_parameter("perm", [128, 128], bf16, isOutput=False)
    masks = nc.declare_dram_parameter("masks", [128, 128], bf16, isOutput=False)
    ident = nc.declare_dram_parameter("ident", [128, 128], bf16, isOutput=False)
    out = nc.declare_dram_parameter("out", [DIM, T], bf16, isOutput=True)
    tap = os.environ.get("KTAP", "")
    dbg = None
    if tap:
        _tap_shapes = {
            "rtok": ([128, 16], f32),
            "qk": ([128, 4, T], bf16),
            "v": ([128, 16, HPC, 65], bf16),
            "avtok": ([128, 16, F], bf16),
            "avall": ([128, 2, T], bf16),
        }
        shp, dt = _tap_shapes[tap]
        dbg = nc.declare_dram_parameter("dbg", shp, dt, isOutput=True)

    Exp = mybir.ActivationFunctionType.Exp
    Sqrt = mybir.ActivationFunctionType.Sqrt
    mult = mybir.AluOpType.mult
    add = mybir.AluOpType.add

    with ExitStack() as ctx:
        tc = ctx.enter_context(tile.TileContext(nc))
        consts = ctx.enter_context(tc.tile_pool(name="consts", bufs=1))
        persist = ctx.enter_context(tc.tile_pool(name="persist", bufs=1))
        work = ctx.enter_context(tc.tile_pool(name="work", bufs=4))
        vecs = ctx.enter_context(tc.tile_pool(name="vecs", bufs=1))

        # ---- constants / inputs ----
        wk_sb = consts.tile([128, KC, F], bf16, tag="wk")
        wq_sb = consts.tile([128, KC, F], bf16, tag="wq")
        wv_sb = consts.tile([128, KC, F], bf16, tag="wv")
        wo_sb = consts.tile([128, 2, DIM], bf16, tag="wo")
        cos_sb = consts.tile([128, T], bf16, tag="cos")
        sin_sb = consts.tile([128, T], bf16, tag="sin")
        perm_sb = consts.tile([128, 128], bf16, tag="perm")
        mask_sb = consts.tile([128, 128], bf16, tag="mask")
        id_sb = consts.tile([128, 128], bf16, tag="ident")
        ones_col = consts.tile([128, 1], bf16, tag="onesc")
        one_f32 = consts.tile([1, 1], f32, tag="onef")
        xT_sb = persist.tile([128, KC, T], bf16, tag="xT")
        xT_r = xT.rearrange("(kc p) t -> p kc t", p=128)
        # wk first (first PE consumer), then xT chunks in consumption order
        # with the other weights slotted behind the early chunks
        nc.sync.dma_start(wk_sb, wk.rearrange("(kc p) f -> p kc f", p=128))
        for kc in KORD[:4]:
            nc.sync.dma_start(xT_sb[:, kc], xT_r[:, kc])
        nc.sync.dma_start(perm_sb, perm[:, :])
        nc.sync.dma_start(cos_sb, cosT[:, :])
        nc.sync.dma_start(sin_sb, sinT[:, :])
        for kc in KORD[4:]:
            nc.sync.dma_start(xT_sb[:, kc], xT_r[:, kc])
        nc.sync.dma_start(wq_sb, wq.rearrange("(kc p) f -> p kc f", p=128))
        nc.sync.dma_start(wv_sb, wv.rearrange("(kc p) f -> p kc f", p=128))
        nc.sync.dma_start(mask_sb, masks[:, :])
        nc.sync.dma_start(id_sb, ident[:, :])
        nc.sync.dma_start(wo_sb, wo.rearrange("(fc p) d -> p fc d", p=128))
        nc.vector.memset(ones_col, 1.0)
        nc.vector.memset(one_f32, 1.0)

        # persistent activations
        qk_sb = persist.tile([128, 4, T], bf16, tag="qk")  # 0,1=q fc0/1; 2,3=k
        v_sb = persist.tile([128, 16, HPC, 65], bf16, tag="v")
        av_tok = persist.tile([128, 16, F], bf16, tag="avtok")
        av_all = persist.tile([128, 2, T], bf16, tag="av")
        r_sb = vecs.tile([1, T], f32, tag="r")
        r_tok = vecs.tile([128, 16], f32, tag="rtok")
        r_bc = persist.tile([128, T], f32, tag="rbc")
        cosr_sb = persist.tile([128, T], bf16, tag="cosr")
        sinr_sb = persist.tile([128, T], bf16, tag="sinr")
        nc.vector.memset(v_sb[:, :, :, 64:65], 1.0)

        ctxA = ExitStack()
        psKQ = ctxA.enter_context(tc.tile_pool(name="psKQ", bufs=8, space="PSUM"))
        sbA = ctxA.enter_context(tc.tile_pool(name="sbA", bufs=1))
        xsq_sb = sbA.tile([128, KC, T], bf16, tag="xsq")

        # x^2 per chunk (DVE, chases the xT DMAs)
        for kc in KORD:
            nc.vector.tensor_mul(xsq_sb[:, kc], xT_sb[:, kc], xT_sb[:, kc])

        def proj_rope(fidx, psum_tiles, is_q):
            """Finish a Q/K projection: PSUM->SBUF copy, rotate-half perm
            matmul, rope multiply-adds into qk_sb[fidx]. Q uses the r-scaled
            tables so r_q rides in for free."""
            cc = cosr_sb if is_q else cos_sb
            ssb = sinr_sb if is_q else sin_sb
            for tt in range(4):
                ts = slice(tt * 512, (tt + 1) * 512)
                raw = work.tile([128, 512], bf16, tag="raw")
                if is_q or tt % 2 == 0:
                    nc.scalar.copy(out=raw, in_=psum_tiles[tt])
                else:
                    nc.vector.tensor_copy(out=raw, in_=psum_tiles[tt])
                pp = psKQ.tile([128, 512], f32, tag="proj", name=f"pp_{fidx}_{tt}")
                nc.tensor.matmul(pp, lhsT=perm_sb, rhs=raw, start=True, stop=True)
                t1 = work.tile([128, 512], bf16, tag="t1")
                nc.vector.tensor_tensor(t1, pp, ssb[:, ts], mult)
                t2 = work.tile([128, 512], bf16, tag="t2")
                nc.vector.tensor_tensor(t2, raw, cc[:, ts], mult)
                if is_q:
                    nc.vector.tensor_tensor(qk_sb[:, fidx, ts], t2, t1, add)
                else:
                    nc.gpsimd.tensor_tensor(qk_sb[:, fidx, ts], t2, t1, add)

        # ---- K projection (both fc), chunk-paced off the xT DMA stream ----
        psK = {}
        for fc in range(2):
            for tt in range(4):
                psK[(fc, tt)] = psKQ.tile(
                    [128, 512], f32, tag="proj", name=f"k_{fc}_{tt}"
                )
        for kc in KORD:
            for fc in range(2):
                for tt in range(4):
                    nc.tensor.matmul(
                        psK[(fc, tt)],
                        lhsT=wk_sb[:, kc, fc * 128 : (fc + 1) * 128],
                        rhs=xT_sb[:, kc, tt * 512 : (tt + 1) * 512],
                        start=(kc == KORD[0]),
                        stop=(kc == KORD[-1]),
                    )
        for fc in range(2):
            proj_rope(2 + fc, [psK[(fc, tt)] for tt in range(4)], False)

        # ---- sum(x^2) ones-matmuls with the r-chain pipelined per slice ----
        ss_sb = sbA.tile([1, T], f32, tag="ss")
        sq_sb = sbA.tile([1, T], f32, tag="sq")
        for s in range(4):
            ts = slice(s * 512, (s + 1) * 512)
            ss_ps = psKQ.tile([1, 512], f32, tag="proj", name=f"ss_{s}")
            for kc in range(KC):
                nc.tensor.matmul(
                    ss_ps,
                    lhsT=ones_col,
                    rhs=xsq_sb[:, kc, s * 512 : (s + 1) * 512],
                    start=(kc == 0),
                    stop=(kc == KC - 1),
                )
            nc.scalar.copy(out=ss_sb[:, ts], in_=ss_ps)
            nc.scalar.activation(sq_sb[:, ts], ss_sb[:, ts], Sqrt, scale=1.0 / DIM)
            nc.vector.reciprocal(r_sb[:, ts], sq_sb[:, ts])
            nc.gpsimd.partition_broadcast(r_bc[:, ts], r_sb[:, ts])
            nc.gpsimd.tensor_tensor(cosr_sb[:, ts], cos_sb[:, ts], r_bc[:, ts], mult)
            nc.gpsimd.tensor_tensor(sinr_sb[:, ts], sin_sb[:, ts], r_bc[:, ts], mult)
        # Q fc0 projection
        psQ0 = [
            psKQ.tile([128, 512], f32, tag="proj", name=f"q0_{tt}")
            for tt in range(4)
        ]
        for kc in range(KC):
            for tt in range(4):
                nc.tensor.matmul(
                    psQ0[tt],
                    lhsT=wq_sb[:, kc, 0:128],
                    rhs=xT_sb[:, kc, tt * 512 : (tt + 1) * 512],
                    start=(kc == 0),
                    stop=(kc == KC - 1),
                )
        # r_tok via PE transposes of the r row
        rtok_ps = psKQ.tile([128, 16], f32, tag="proj", name="rtokps")
        for i in range(16):
            nc.tensor.transpose(
                rtok_ps[:, i : i + 1], r_sb[0:1, i * 128 : (i + 1) * 128],
                one_f32,
            )
        nc.vector.tensor_copy(out=r_tok, in_=rtok_ps)

        proj_rope(0, psQ0, True)

        # ---- Q fc1 ----
        psQ1 = [
            psKQ.tile([128, 512], f32, tag="proj", name=f"q1_{tt}")
            for tt in range(4)
        ]
        for kc in range(KC):
            for tt in range(4):
                nc.tensor.matmul(
                    psQ1[tt],
                    lhsT=wq_sb[:, kc, 128:256],
                    rhs=xT_sb[:, kc, tt * 512 : (tt + 1) * 512],
                    start=(kc == 0),
                    stop=(kc == KC - 1),
                )
        proj_rope(1, psQ1, True)

        # ---- V projection (token-major) + r_tok scaling ----
        ctxA.close()

        # ---- attention: scores [k,q] -> paired exp -> transposed AV ----
        # PSUM: sc ring (2x2 banks, also V-proj) + av4/po ring (3) + avT (1)
        with (
            tc.tile_pool(name="psSC", bufs=3, space="PSUM") as psSC,
            tc.tile_pool(name="psAV", bufs=2, space="PSUM") as psAV,
            tc.tile_pool(name="expp", bufs=6) as expp,
            tc.tile_pool(name="recp", bufs=4) as recp,
        ):
            def v_proj(tt):
                psv = psSC.tile([128, 256], f32, tag="sc", name=f"v_{tt}")
                for kc in range(KC):
                    nc.tensor.matmul(
                        psv,
                        lhsT=xT_sb[:, kc, tt * 128 : (tt + 1) * 128],
                        rhs=wv_sb[:, kc, :],
                        start=(kc == 0),
                        stop=(kc == KC - 1),
                    )
                nc.vector.tensor_scalar(
                    out=v_sb[:, tt, :, 0:64],
                    in0=psv.rearrange("p (h d) -> p h d", h=HPC),
                    scalar1=r_tok[:, tt : tt + 1],
                    scalar2=None,
                    op0=mult,
                )

            def emit_outproj_do(qtp, do):
                po = psSC.tile([128, 512], f32, tag="sc", name=f"o_{qtp}_{do}")
                for fc in range(2):
                    nc.tensor.matmul(
                        po,
                        lhsT=wo_sb[:, fc, do * 128 : (do + 1) * 128],
                        rhs=av_all[:, fc, qtp * 512 : (qtp + 1) * 512],
                        start=(fc == 0),
                        stop=(fc == 1),
                    )
                ob = work.tile([128, 512], bf16, tag="ob")
                nc.vector.tensor_copy(out=ob, in_=po)
                nc.sync.dma_start(
                    out.rearrange("(do p) t -> p do t", p=128)[
                        :, do, qtp * 512 : (qtp + 1) * 512
                    ],
                    ob,
                )

            pending_oq = None  # (qt_prev, next_do)
            for qt in range(4):
                q0 = qt * 512
                for pi in range(2):
                    # full-bank tiles: matmul start=True zeroes the whole 2KB
                    # bank, so only the FIRST write into each bank uses it
                    av4 = [
                        psAV.tile([128, 4, 128], f32, tag="av4",
                                  name=f"av_{qt}_{pi}_{x}")
                        for x in range(2)
                    ]
                    def emit_av(kb, ex):
                        for qbl in range(4):
                            qb = 4 * qt + qbl
                            if kb > qb:
                                continue
                            for x in range(2):
                                nc.tensor.matmul(
                                    av4[x][:, qbl, 0:65],
                                    lhsT=ex[:, x * 512 + qbl * 128 : x * 512 + (qbl + 1) * 128],
                                    rhs=v_sb[:, kb, 2 * pi + x, :],
                                    start=(kb == 0 and qbl == 0),
                                    stop=(kb == qb),
                                    skip_group_check=True,
                                )

                    nkb = 4 * qt + 4
                    pend = None
                    for kb in range(nkb + 1):
                        cur = None
                        if kb < nkb:
                            # stagger this quarter's V projections into pair
                            # 0's kb loop so PE fills gaps while Act runs exp
                            if pi == 0 and qt == 0:
                                v_proj(kb)
                            c0 = max(0, kb * 128 - q0)
                            sc = psSC.tile(
                                [128, 1024], f32, tag="sc",
                                name=f"sc_{qt}_{pi}_{kb}"
                            )
                            for x in range(2):
                                rX = slice(x * 64, x * 64 + 64)
                                nc.tensor.matmul(
                                    sc[:, x * 512 + c0 : x * 512 + 512],
                                    lhsT=qk_sb[rX, 2 + pi, kb * 128 : (kb + 1) * 128],
                                    rhs=qk_sb[rX, pi, q0 + c0 : q0 + 512],
                                    start=True,
                                    stop=True,
                                )
                            if pi == 0 and qt > 0 and kb < 4:
                                v_proj(4 * qt + kb)
                            # spread the previous quarter's out-projection
                            # over this kb loop to keep Act fed with scores
                            if pi == 0 and pending_oq is not None and kb >= 2:
                                qtp, nd = pending_oq
                                todo = 8 - nd
                                left = nkb - kb
                                n_emit = -(-todo // max(left, 1))
                                for _ in range(min(n_emit, todo)):
                                    emit_outproj_do(qtp, nd)
                                    nd += 1
                                pending_oq = (qtp, nd) if nd < 8 else None
                            ex = expp.tile([128, 1024], bf16, tag="exp")
                            if c0 == 0:
                                nc.scalar.activation(
                                    ex, sc, Exp, scale=r_tok[:, kb : kb + 1]
                                )
                            else:
                                for x in range(2):
                                    nc.scalar.activation(
                                        ex[:, x * 512 + c0 : x * 512 + 512],
                                        sc[:, x * 512 + c0 : x * 512 + 512],
                                        Exp,
                                        scale=r_tok[:, kb : kb + 1],
                                    )
                            if kb >= 4 * qt:  # diagonal block: causal mask
                                for x in range(2):
                                    nc.gpsimd.tensor_tensor(
                                        ex[:, x * 512 + c0 : x * 512 + c0 + 128],
                                        ex[:, x * 512 + c0 : x * 512 + c0 + 128],
                                        mask_sb,
                                        mult,
                                    )
                            cur = (kb, ex)
                        if pend is not None:
                            emit_av(*pend)
                        pend = cur
                    # normalize (rows 0..63 / row 64) into token-major av_tok
                    for x in range(2):
                        h = 2 * pi + x
                        rec4 = recp.tile([128, 4], f32, tag="rec")
                        nc.vector.reciprocal(rec4, av4[x][:, :, 64:65])
                        for qbl in range(4):
                            nc.vector.tensor_scalar(
                                out=av_tok[:, 4 * qt + qbl, h * 64 : (h + 1) * 64],
                                in0=av4[x][:, qbl, 0:64],
                                scalar1=rec4[:, qbl : qbl + 1],
                                scalar2=None,
                                op0=mult,
                            )
                # back to feature-major via PE transposes (53ns each)
                avT = psSC.tile([128, 8, 128], bf16, tag="sc", name=f"avt_{qt}")
                for j, tt in enumerate(range(4 * qt, 4 * qt + 4)):
                    for fc in range(2):
                        nc.tensor.transpose(
                            avT[:, fc * 4 + j, :],
                            av_tok[:, tt, fc * 128 : (fc + 1) * 128],
                            id_sb,
                        )
                for fc in range(2):
                    nc.vector.tensor_copy(
                        out=av_all[:, fc, q0 : q0 + 512],
                        in_=avT[:, fc * 4 : fc * 4 + 4, :],
                    )
                # out-projection: deferred into the next quarter's kb loop
                # (the last quarter runs immediately as the tail)
                if qt < 3:
                    pending_oq = (qt, 0)
                else:
                    for do in range(8):
                        emit_outproj_do(3, do)
            if tap == "rtok":
                nc.sync.dma_start(dbg[:, :], r_tok)
            elif tap == "qk":
                nc.sync.dma_start(dbg[:, :, :], qk_sb)
            elif tap == "v":
                nc.sync.dma_start(dbg[:, :, :, :], v_sb)
            elif tap == "avtok":
                nc.sync.dma_start(dbg[:, :, :], av_tok)
            elif tap == "avall":
                nc.sync.dma_start(dbg[:, :, :], av_all)
    nc.compile()
    return nc


def _host_inputs(x, norm_w, w_qkv, w_o, sin, cos):
    """Build the 8 per-core input maps (all bf16)."""
    n = T
    w_eff = np.asarray(w_qkv, np.float64) * np.asarray(norm_w, np.float64)[:, None]
    sin_n = np.asarray(sin, np.float32)[:n]  # [T, 64]
    cos_n = np.asarray(cos, np.float32)[:n]
    sign = np.concatenate([-np.ones(32, np.float32), np.ones(32, np.float32)])
    cos_tile = np.tile(cos_n.T, (2, 1))  # [128, T]
    sin_tile = np.tile((sin_n * sign[None, :]).T, (2, 1))  # [128, T]
    perm = np.zeros((128, 128), np.float32)
    for m in range(128):
        d = m % 64
        k = m + 32 if d < 32 else m - 32
        perm[k, m] = 1.0
    ident_np = np.eye(128, dtype=np.float32)
    ql = np.arange(128)[None, :]
    key = np.arange(128)[:, None]
    masks = (ql >= key).astype(np.float32)

    in_maps = []
    for c in range(8):
        b, g = c // 4, c % 4
        fs = slice(g * F, (g + 1) * F)
        in_maps.append(
            {
                "xT": np.ascontiguousarray(np.asarray(x, np.float32)[b].T).astype(BF16),
                "wq": (w_eff[:, 0:DIM][:, fs] * (DIM_HEAD ** -0.5)).astype(BF16),
                "wk": w_eff[:, DIM : 2 * DIM][:, fs].astype(BF16),
                "wv": w_eff[:, 2 * DIM : 3 * DIM][:, fs].astype(BF16),
                "wo": np.asarray(w_o, np.float32)[fs, :].astype(BF16),
                "cosT": cos_tile.astype(BF16),
                "sinT": sin_tile.astype(BF16),
                "perm": perm.astype(BF16),
                "masks": masks.astype(BF16),
                "ident": ident_np.astype(BF16),
            }
        )
    return in_maps


def kernel(x, norm_w, w_qkv, w_o, b_o, sin, cos):
    from concourse.bass_utils import run_bass_kernel_spmd

    if "nc" not in _NC_CACHE:
        _NC_CACHE["nc"] = _build_nc()
    nc = _NC_CACHE["nc"]
    in_maps = _host_inputs(x, norm_w, w_qkv, w_o, sin, cos)
    trace = bool(int(os.environ.get("KERNEL_TRACE", "0")))
    res = run_bass_kernel_spmd(nc, in_maps, core_ids=list(range(8)), trace=trace)
    if trace and res.exec_time_ns is not None:
        print(f"HW exec time: {res.exec_time_ns} ns")
    outs = [r["out"].astype(np.float32) for r in res.results]  # [1024, T] fm
    b_o = np.asarray(b_o, np.float32)
    full = np.empty((B, T, DIM), np.float32)
    for b in range(B):
        acc = outs[b * 4] + outs[b * 4 + 1] + outs[b * 4 + 2] + outs[b * 4 + 3]
        full[b] = acc.T + b_o[None, :]
    return full
